# revision 13
# baseline (speedup 1.0000x reference)
"""Trainium2 Bass kernel for nn_AttentionModel (graph attention encoder + decoder).

Contract: kernel(**inputs) takes FULL unsharded numpy inputs (as produced by
reference.setup_inputs()) and returns the FULL [256, 100] float32 output.
Internally shards the batch (256) across 8 NeuronCores (32 each, pure data
parallel; weights replicated) and runs a fused Bass/Tile kernel per core.

v3: folded weights (Wq@Wk^T, Wv@Wo), transposed-score attention (no score
transposes), pow-based layernorm rstd (single act table), host-pretransposed
fp16 observation, matmul-based decoder, and N-way software pipelining across
batch elements to keep all engines fed.

Self-contained: hardcodes all shapes; no sibling imports.
"""

import sys

for _p in ("/opt/trn_rl_repo", "/opt/pypackages"):
    if _p not in sys.path:
        sys.path.append(_p)

import numpy as np
from contextlib import ExitStack

# --- static architecture constants ---
B, IH, IL, LH, E, FFH, NL = 256, 200, 6, 100, 256, 512, 2
G = IH + LH + 1  # 301
CLIP = 10.0
SCALE = 1.0 / 16.0  # 1/sqrt(E)
NCORES = 8
BPC = B // NCORES  # 32 batch elements per core
NWAY = 2  # software pipeline lanes

GC = [(0, 128), (128, 256), (256, 301)]   # g chunks (written ranges)
GC2 = [(0, 128), (128, 256), (256, 302)]  # even-padded q ranges for rsum
VN = 302  # padded moving width over the node axis
SEGS = [(0, IH, "i"), (IH, IH + LH, "l"), (IH + LH, G, "n")]  # embed type slices


# ----------------------------------------------------------------------------
# host-side weight packing
# ----------------------------------------------------------------------------
def _tf32(x):
    """Round fp32 array to tfloat32 (10 mantissa bits), round-to-nearest-even."""
    u = np.ascontiguousarray(x, np.float32).view(np.uint32)
    u = (u + 0x0FFF + ((u >> 13) & 1)) & np.uint32(0xFFFFE000)
    return u.view(np.float32)


def _pack_rows(m, nchunk):
    """[nchunk*128, N] -> [128, nchunk, N] with [:, k, :] = m[128k:128(k+1), :]"""
    return np.ascontiguousarray(
        np.stack([m[i * 128:(i + 1) * 128] for i in range(nchunk)], axis=1)
    ).astype(np.float32)


def _prep_weights(inp):
    w = {}
    # --- embedding (fp16) ---
    for t, wk1, bk1, wk2, bk2 in (
        ("i", "wi1", "bi1", "wi2", "bi2"),
        ("l", "wl1", "bl1", "wl2", "bl2"),
        ("n", "wn1", "bn1", "wn2", "bn2"),
    ):
        w[f"w1_{t}"] = np.asarray(inp[wk1], np.float16)
        w[f"b1r_{t}"] = np.asarray(inp[bk1], np.float16).reshape(1, 32)
        w[f"w2_{t}"] = np.asarray(inp[wk2], np.float16)           # [32, 256]
        w[f"b2r_{t}"] = np.asarray(inp[bk2], np.float16).reshape(1, E)
    # --- encoder layers (f32r) ---
    for l in range(NL):
        wqk = (np.asarray(inp["enc_wq"][l], np.float64) * SCALE) @ np.asarray(
            inp["enc_wk"][l], np.float64).T
        wvo = np.asarray(inp["enc_wv"][l], np.float64) @ np.asarray(
            inp["enc_wo"][l], np.float64)
        w[f"wqk{l}"] = _tf32(_pack_rows(wqk.astype(np.float32), 2))   # lhsT chunks
        w[f"wvo{l}"] = _tf32(_pack_rows(wvo.astype(np.float32), 2))   # rhs chunks
        w[f"wf1{l}"] = _tf32(_pack_rows(inp["enc_wf1"][l], 2))        # [128,2,512] lhsT
        w[f"bf1{l}"] = np.ascontiguousarray(
            np.asarray(inp["enc_bf1"][l], np.float32).reshape(4, 128).T)  # [128, 4]
        w[f"wf2{l}"] = _tf32(_pack_rows(inp["enc_wf2"][l], 4))        # [128,4,256] rhs
        w[f"bf2r{l}"] = _tf32(np.asarray(inp["enc_bf2"][l], np.float32).reshape(1, E))
    # decoder fused matrix: compat = h_leaf . (M @ ge), M = Wpn_E @ Wfc.T
    MT = (np.asarray(inp["w_fc"], np.float64) @ np.asarray(
        inp["w_pn"], np.float64)[:, :E].T) * SCALE
    w["mt"] = _tf32(_pack_rows(MT.astype(np.float32), 2))             # [128,2,256] lhsT
    return w


def _prep_obs(observation):
    """Per-core observation-derived arrays (host-side layout transforms)."""
    obs = np.asarray(observation, np.float32)
    nb = obs.shape[0]
    # transposed features, fp16, g padded 301->304 with zeros
    xt = np.zeros((nb, 9, 304), np.float32)
    xt[:, :, :G] = obs.transpose(0, 2, 1).astype(np.float16).astype(np.float32)
    # mask columns [nb, 128, 3]: mcol[b, p, t] = mask[b, 128t + p]
    mask = obs[:, :, 8]
    mpad = np.zeros((nb, 384), np.float32)
    mpad[:, :G] = mask
    mcol = np.repeat(mpad.reshape(nb, 3, 128).transpose(0, 2, 1)[..., None], 2, axis=3)
    mcol = np.ascontiguousarray(mcol)
    # batched tail masks
    maskbt = np.zeros((nb, 304), np.float32)
    maskbt[:, :G] = mask
    lvbt = np.ascontiguousarray(mask[:, IH:IH + LH])
    return {"xt": xt, "mcol": mcol, "maskbt": maskbt, "lvbt": lvbt}


# ----------------------------------------------------------------------------
# numpy mirror of the device computation (for algebra validation)
# ----------------------------------------------------------------------------
def _numpy_mirror(observation, w):
    obs = np.asarray(observation, np.float32)
    nb = obs.shape[0]
    out = np.zeros((nb, LH), np.float32)

    def lrelu(x):
        return np.maximum(x, 0.01 * x)

    def ln(x):
        m = x.mean(-1, keepdims=True)
        v = x.var(-1, keepdims=True)
        return (x - m) * (v + 1e-5) ** -0.5

    for b in range(nb):
        xT = obs[b].T.astype(np.float16).astype(np.float32)  # [9, 301]
        h = np.zeros((G, E), np.float32)
        for (c0, c1, ty), nf in zip(SEGS, (6, 8, 6)):
            z = xT[:nf, c0:c1].T @ w[f"w1_{ty}"].astype(np.float32) \
                + w[f"b1r_{ty}"].astype(np.float32)
            h[c0:c1] = lrelu(z) @ w[f"w2_{ty}"].astype(np.float32) \
                + w[f"b2r_{ty}"].astype(np.float32)

        for l in range(NL):
            wqk = np.concatenate([w[f"wqk{l}"][:, 0], w[f"wqk{l}"][:, 1]], 0)
            wvo = np.concatenate([w[f"wvo{l}"][:, 0], w[f"wvo{l}"][:, 1]], 0)
            wf1 = np.concatenate([w[f"wf1{l}"][:, 0], w[f"wf1{l}"][:, 1]], 0)
            wf2 = np.concatenate([w[f"wf2{l}"][:, k] for k in range(4)], 0)
            bf1 = w[f"bf1{l}"].T.reshape(-1)
            u = h @ wqk
            sT = h @ u.T                      # sT[k, q]
            aT = np.exp(sT)
            rs = aT.sum(0)                    # per q
            o = (aT.T @ h) @ wvo              # [q, e]
            x1 = o / rs[:, None] + h
            h1 = ln(x1)
            p = np.maximum(h1 @ wf1 + bf1, 0.0)
            x2 = p @ wf2 + w[f"bf2r{l}"] + h1
            h = ln(x2)

        mask = obs[b, :, 8]
        ge = (h * mask[:, None]).sum(0)       # unnormalized
        MT = np.concatenate([w["mt"][:, 0], w["mt"][:, 1]], 0)
        c = MT.T @ ge
        compat = h[IH:IH + LH] @ c            # unmasked leaf compat
        vlen = mask.sum()
        logits = np.tanh(compat / vlen) * CLIP
        ee = np.exp(logits)
        pp = ee / ee.sum()
        lv = mask[IH:IH + LH]
        masked = pp * lv + 1e-20
        out[b] = masked / masked.sum()
    return out


# ----------------------------------------------------------------------------
# the Bass/Tile kernel
# ----------------------------------------------------------------------------
def _build(bpc, nbp=None, dbg=False):
    import concourse.bass as bass
    import concourse.mybir as mybir
    import concourse.tile as tile
    from concourse import bacc
    from concourse.masks import make_identity

    f32 = mybir.dt.float32
    f32r = mybir.dt.float32r
    f16 = mybir.dt.float16
    AF = mybir.ActivationFunctionType
    ALU = mybir.AluOpType
    AX = mybir.AxisListType

    nc = bacc.Bacc(None, target_bir_lowering=False)

    xt_d = nc.declare_dram_parameter("xt", [bpc, 9, 304], f32, isOutput=False)
    mcol_d = nc.declare_dram_parameter("mcol", [bpc, 128, 3, 2], f32r, isOutput=False)
    maskbt_d = nc.declare_dram_parameter("maskbt", [bpc, 304], f32, isOutput=False)
    lvbt_d = nc.declare_dram_parameter("lvbt", [bpc, LH], f32, isOutput=False)
    dp = {}
    for t in "iln":
        nf = 8 if t == "l" else 6
        dp[f"w1_{t}"] = nc.declare_dram_parameter(f"w1_{t}", [nf, 32], f16, isOutput=False)
        dp[f"b1r_{t}"] = nc.declare_dram_parameter(f"b1r_{t}", [1, 32], f16, isOutput=False)
        dp[f"w2_{t}"] = nc.declare_dram_parameter(f"w2_{t}", [32, E], f16, isOutput=False)
        dp[f"b2r_{t}"] = nc.declare_dram_parameter(f"b2r_{t}", [1, E], f16, isOutput=False)
    for l in range(NL):
        dp[f"wqk{l}"] = nc.declare_dram_parameter(f"wqk{l}", [128, 2, E], f32r, isOutput=False)
        dp[f"wvo{l}"] = nc.declare_dram_parameter(f"wvo{l}", [128, 2, E], f32r, isOutput=False)
        dp[f"wf1{l}"] = nc.declare_dram_parameter(f"wf1{l}", [128, 2, FFH], f32r, isOutput=False)
        dp[f"bf1{l}"] = nc.declare_dram_parameter(f"bf1{l}", [128, 4], f32, isOutput=False)
        dp[f"wf2{l}"] = nc.declare_dram_parameter(f"wf2{l}", [128, 4, E], f32r, isOutput=False)
        dp[f"bf2r{l}"] = nc.declare_dram_parameter(f"bf2r{l}", [1, E], f32r, isOutput=False)
    dp["mt"] = nc.declare_dram_parameter("mt", [128, 2, E], f32r, isOutput=False)
    out_d = nc.declare_dram_parameter("out", [bpc, LH], f32, isOutput=True)
    if dbg:
        dbg_hT = nc.declare_dram_parameter("dbg_hT", [128, 2, 304], f32, isOutput=True)
        dbg_hnat = nc.declare_dram_parameter("dbg_hnat", [128, 3, E], f32, isOutput=True)
        dbg_aT = nc.declare_dram_parameter("dbg_aT", [128, 3, 304], f32, isOutput=True)
        dbg_rinv = nc.declare_dram_parameter("dbg_rinv", [128, 3], f32, isOutput=True)
        dbg_x1 = nc.declare_dram_parameter("dbg_x1", [128, 3, E], f32, isOutput=True)
        dbg_h1 = nc.declare_dram_parameter("dbg_h1", [128, 3, E], f32, isOutput=True)
        dbg_hl0 = nc.declare_dram_parameter("dbg_hl0", [128, 3, E], f32, isOutput=True)
        dbg_z1 = nc.declare_dram_parameter("dbg_z1", [32, 304], f32, isOutput=True)
        dbg_lr = nc.declare_dram_parameter("dbg_lr", [32, 304], f32, isOutput=True)
        dbg_xT = nc.declare_dram_parameter("dbg_xT", [9, 304], f32, isOutput=True)
        dbg_ge = nc.declare_dram_parameter("dbg_ge", [128, 2, 2], f32, isOutput=True)
        dbg_c = nc.declare_dram_parameter("dbg_c", [128, 2, 2], f32, isOutput=True)
        dbg_cc = nc.declare_dram_parameter("dbg_cc", [LH, 2], f32, isOutput=True)

    with tile.TileContext(nc) as tc, ExitStack() as ctx:
        const = ctx.enter_context(tc.tile_pool(name="const", bufs=1))
        st = ctx.enter_context(tc.tile_pool(name="st", bufs=2))
        ps = ctx.enter_context(tc.tile_pool(name="ps", bufs=6, space="PSUM"))

        # ---- constants / weights into SBUF ----
        ident = const.tile([128, 128], f32, tag="ident")
        make_identity(nc, ident)
        ident_r = const.tile([128, 128], f32r, tag="ident_r")
        nc.vector.tensor_copy(out=ident_r, in_=ident)
        ones_f = const.tile([1, 512], f32, tag="ones_f")
        nc.vector.memset(ones_f, 1.0)
        ones_r = const.tile([1, 512], f32r, tag="ones_r")
        nc.vector.tensor_copy(out=ones_r, in_=ones_f)
        ones16 = const.tile([1, 512], f16, tag="ones16")
        nc.vector.tensor_copy(out=ones16, in_=ones_f)
        eps = const.tile([128, 1], f32, tag="eps")
        nc.vector.memset(eps, 1e-5)
        onecol_f = const.tile([128, 2], f32, tag="onecol_f")
        nc.vector.memset(onecol_f, 1.0)
        onecol2_r = const.tile([128, 2], f32r, tag="onecol2_r")
        nc.vector.tensor_copy(out=onecol2_r, in_=onecol_f)

        cw = {}
        for nm, h in dp.items():
            t = const.tile(list(h.shape), h.dtype, tag=f"w_{nm}")
            nc.sync.dma_start(out=t, in_=h[:])
            cw[nm] = t

        mask_bt = const.tile([bpc, 304], f32, tag="mask_bt")
        nc.sync.dma_start(out=mask_bt, in_=maskbt_d[:])
        lv_bt = const.tile([bpc, LH], f32, tag="lv_bt")
        nc.sync.dma_start(out=lv_bt, in_=lvbt_d[:])

        compat_cols = const.tile([LH, max(bpc, 2)], f32, tag="compat_cols")

        def mm(out, lhsT, rhs, start, stop):
            nc.tensor.matmul(out, lhsT, rhs, start=start, stop=stop)

        ENGS3 = (nc.scalar, nc.vector)

        def ecopy(eng, out, in_):
            if eng is nc.scalar:
                nc.scalar.copy(out=out, in_=in_)
            else:
                eng.tensor_copy(out=out, in_=in_)

        # ================= per batch element (generator, staged) =============
        def elem(b, lane):
            L = str(lane)
            rr = [0]  # rotating engine picker for copies

            def pick():
                e = ENGS3[rr[0] % 2]
                rr[0] += 1
                return e

            # ---- S0: loads ----
            xT32 = st.tile([9, 304], f32, tag="xT32" + L, name="xT32")
            nc.sync.dma_start(out=xT32, in_=xt_d[b])
            xT = st.tile([9, 304], f16, tag="xT" + L, name="xT")
            nc.vector.tensor_copy(out=xT, in_=xT32)
            mcol = st.tile([128, 3, 2], f32r, tag="mcol" + L, name="mcol")
            nc.sync.dma_start(out=mcol, in_=mcol_d[b])
            yield

            # ---- S1: z1 = W1^T x + b1 ; lr = leakyrelu(z1) ----
            z1 = ps.tile([32, VN], f32, tag="ps", name="z1")
            for (c0, c1, ty), nf in zip(SEGS, (6, 8, 6)):
                mm(z1[:, c0:c1], cw[f"w1_{ty}"], xT[:nf, c0:c1], True, False)
                mm(z1[:, c0:c1], cw[f"b1r_{ty}"], ones16[:, :c1 - c0], False, True)
            if dbg and b == 0:
                z1c = st.tile([32, 304], f32, tag="z1c" + L, name="z1c")
                nc.vector.tensor_copy(out=z1c[:, :G], in_=z1[:, :G])
                nc.sync.dma_start(out=dbg_z1[:], in_=z1c)
                xTc = st.tile([9, 304], f32, tag="xTc" + L, name="xTc")
                nc.vector.tensor_copy(out=xTc, in_=xT)
                nc.sync.dma_start(out=dbg_xT[:], in_=xTc)
            small1 = st.tile([32, 304], f16, tag="small1" + L, name="small1")
            nc.scalar.activation(small1[:, :G], z1[:, :G], AF.Identity, scale=0.01)
            lr = st.tile([32, 304], f16, tag="lr" + L, name="lr")
            nc.vector.tensor_tensor(out=lr[:, :G], in0=z1[:, :G], in1=small1[:, :G],
                                    op=ALU.max)
            yield

            if dbg and b == 0:
                lrc = st.tile([32, 304], f32, tag="lrc" + L, name="lrc")
                nc.vector.tensor_copy(out=lrc, in_=lr)
                nc.sync.dma_start(out=dbg_lr[:], in_=lrc)
            # ---- S2: h0T[e, g] = W2^T lr + b2 (direct transposed embed) ----
            hT = st.tile([128, 2, 304], f32r, tag="hT" + L, name="hT")
            for m in range(2):
                hp = ps.tile([128, VN], f32, tag="ps", name=f"h0p{m}")
                for si, (c0, c1, ty) in enumerate(SEGS):
                    mm(hp[:, c0:c1], cw[f"w2_{ty}"][:, m * 128:(m + 1) * 128],
                       lr[:, c0:c1], True, False)
                    mm(hp[:, c0:c1], cw[f"b2r_{ty}"][:, m * 128:(m + 1) * 128],
                       ones16[:, :c1 - c0], False, True)
                ecopy(pick(), hT[:, m, :G], hp[:, :G])
            yield

            # ---- S3: h_nat = transpose(hT) ----
            h_nat = st.tile([128, 3, E], f32r, tag="hnat" + L, name="h_nat")
            for t in range(3):
                g0, g1 = GC2[t]
                gs = GC[t][1] - g0
                tpn = ps.tile([128, E], f32r, tag="ps", name=f"tpn{t}")
                for m in range(2):
                    nc.tensor.transpose(tpn[:g1 - g0, m * 128:(m + 1) * 128],
                                        hT[:, m, g0:g1], ident_r)
                ecopy(pick(), h_nat[:gs, t, :], tpn[:gs, :])
            yield

            if dbg and b == 0:
                nc.sync.dma_start(out=dbg_hT[:], in_=hT.bitcast(f32))
                nc.sync.dma_start(out=dbg_hnat[:], in_=h_nat.bitcast(f32))
            # ================= encoder layers =================
            for l in range(NL):
                # ---- S4: uT[e', g] = WQK^T hT ----
                u = st.tile([128, 2, 304], f32r, tag="u" + L, name="u")
                for m in range(2):
                    up = ps.tile([128, VN], f32, tag="ps", name=f"up{m}")
                    for k in range(2):
                        mm(up, cw[f"wqk{l}"][:, k, m * 128:(m + 1) * 128],
                           hT[:, k, :VN], k == 0, k == 1)
                    ecopy(pick(), u[:, m, :VN], up)
                yield

                # ---- S5: sT[k, q] = hT^T u ; aT = exp(sT) ----
                aT = st.tile([128, 3, 304], f32r, tag="aT" + L, name="aT")
                for t in range(3):
                    g0, g1 = GC[t]
                    gs = g1 - g0
                    sp = ps.tile([128, VN], f32, tag="ps", name=f"sp{t}")
                    for m in range(2):
                        mm(sp[:gs, :], hT[:, m, g0:g1], u[:, m, :VN], m == 0, m == 1)
                    nc.scalar.activation(aT[:gs, t, :VN], sp[:gs, :VN], AF.Exp)
                yield

                # ---- S6: rsum per q (column form) ; rinv = 1/rsum ----
                rs = ps.tile([128, 6], f32, tag="psS", name="rs", bufs=2)
                for tq in range(3):
                    q0, q1 = GC2[tq]
                    for tk in range(3):
                        k0, k1 = GC[tk]
                        ks = k1 - k0
                        mm(rs[:q1 - q0, 2 * tq:2 * tq + 2], aT[:ks, tk, q0:q1],
                           onecol2_r[:ks], tk == 0, tk == 2)
                rinv = st.tile([128, 3], f32, tag="rinv" + L, name="rinv")
                for tq in range(3):
                    q0, q1 = GC2[tq]
                    nc.vector.reciprocal(rinv[:q1 - q0, tq:tq + 1],
                                         rs[:q1 - q0, 2 * tq:2 * tq + 1])

                if dbg and b == 0 and l == 0:
                    nc.sync.dma_start(out=dbg_aT[:], in_=aT.bitcast(f32))
                    nc.sync.dma_start(out=dbg_rinv[:], in_=rinv)
                # ---- S7: avT[e, q] = h_nat^T aT ----
                avT = st.tile([128, 2, 304], f32r, tag="avT" + L, name="avT")
                for m in range(2):
                    ap_ = ps.tile([128, VN], f32, tag="ps", name=f"avp{m}")
                    for tk in range(3):
                        k0, k1 = GC[tk]
                        ks = k1 - k0
                        mm(ap_, h_nat[:ks, tk, m * 128:(m + 1) * 128],
                           aT[:ks, tk, :VN], tk == 0, tk == 2)
                    ecopy(pick(), avT[:, m, :VN], ap_)
                yield

                # ---- S8: o = av @ WVO ; x1 = o*rinv + h (fused) ----
                x1 = st.tile([128, 3, E], f32, tag="x1" + L, name="x1")
                for t in range(3):
                    g0, g1 = GC[t]
                    gs = g1 - g0
                    op_ = ps.tile([128, E], f32, tag="ps", name=f"op{t}")
                    for k in range(2):
                        mm(op_[:gs, :], avT[:, k, g0:g1], cw[f"wvo{l}"][:, k, :],
                           k == 0, k == 1)
                    nc.vector.scalar_tensor_tensor(
                        out=x1[:gs, t, :], in0=op_[:gs, :],
                        scalar=rinv[:gs, t:t + 1], in1=h_nat[:gs, t, :],
                        op0=ALU.mult, op1=ALU.add)
                yield

                if dbg and b == 0 and l == 0:
                    nc.sync.dma_start(out=dbg_x1[:], in_=x1)
                # ---- S9: LN1 stats ----
                mv1 = st.tile([128, 3, 2], f32, tag="mv1" + L, name="mv1")
                rstd1 = st.tile([128, 3], f32, tag="rstd1" + L, name="rstd1")
                for t in range(3):
                    gs = GC[t][1] - GC[t][0]
                    st6 = st.tile([128, 6], f32, tag="st6" + L, name="st6", bufs=3)
                    nc.vector.bn_stats(out=st6[:gs], in_=x1[:gs, t, :])
                    nc.vector.bn_aggr(out=mv1[:gs, t, :], in_=st6[:gs])
                lnv1 = st.tile([128, 3], f32, tag="lnv1" + L, name="lnv1")
                nc.scalar.activation(lnv1, mv1[:, :, 1], AF.Ln, bias=eps)
                nc.scalar.activation(rstd1, lnv1, AF.Exp, scale=-0.5)
                yield

                # ---- S10: h1 = (x1 - m1) * rstd1 ----
                h1 = st.tile([128, 3, E], f32r, tag="h1" + L, name="h1")
                for t in range(3):
                    gs = GC[t][1] - GC[t][0]
                    nc.gpsimd.tensor_scalar(
                        out=h1[:gs, t, :], in0=x1[:gs, t, :],
                        scalar1=mv1[:gs, t, 0:1], scalar2=rstd1[:gs, t:t + 1],
                        op0=ALU.subtract, op1=ALU.mult)
                yield

                if dbg and b == 0 and l == 0:
                    nc.sync.dma_start(out=dbg_h1[:], in_=h1.bitcast(f32))
                # ---- S11: h1T = transpose(h1) ----
                h1T = st.tile([128, 2, 304], f32r, tag="h1T" + L, name="h1T")
                for m in range(2):
                    tph = ps.tile([128, VN], f32r, tag="ps", name=f"tph{m}")
                    for t in range(3):
                        g0, g1 = GC2[t]
                        gs = g1 - g0
                        nc.tensor.transpose(tph[:, g0:g1],
                                            h1[:gs, t, m * 128:(m + 1) * 128],
                                            ident_r[:gs, :gs])
                    ecopy(pick(), h1T[:, m, :G], tph[:, :G])
                yield

                # ---- S12: f1 = relu(Wf1^T h1T + bf1) ----
                p = st.tile([128, 4, 304], f32r, tag="p" + L, name="p")
                for m in range(4):
                    fp = ps.tile([128, VN], f32, tag="ps", name=f"fp{m}")
                    for k in range(2):
                        mm(fp, cw[f"wf1{l}"][:, k, m * 128:(m + 1) * 128],
                           h1T[:, k, :VN], k == 0, k == 1)
                    nc.scalar.activation(p[:, m, :G], fp[:, :G], AF.Relu,
                                         bias=cw[f"bf1{l}"][:, m:m + 1])
                yield

                # ---- S13: f2 + bf2 ; x2 = f2 + h1 (fused add) ----
                x2 = st.tile([128, 3, E], f32, tag="x2" + L, name="x2")
                for t in range(3):
                    g0, g1 = GC[t]
                    gs = g1 - g0
                    f2p = ps.tile([128, E], f32, tag="ps", name=f"f2p{t}")
                    for m in range(4):
                        mm(f2p[:gs, :], p[:, m, g0:g1], cw[f"wf2{l}"][:, m, :],
                           m == 0, False)
                    mm(f2p[:gs, :], ones_r[:, :gs], cw[f"bf2r{l}"], False, True)
                    nc.vector.tensor_tensor(out=x2[:gs, t, :], in0=f2p[:gs, :],
                                      in1=h1[:gs, t, :], op=ALU.add)
                yield

                # ---- S14: LN2 stats ----
                mv2 = st.tile([128, 3, 2], f32, tag="mv2" + L, name="mv2")
                rstd2 = st.tile([128, 3], f32, tag="rstd2" + L, name="rstd2")
                for t in range(3):
                    gs = GC[t][1] - GC[t][0]
                    st6b = st.tile([128, 6], f32, tag="st6b" + L, name="st6b", bufs=3)
                    nc.vector.bn_stats(out=st6b[:gs], in_=x2[:gs, t, :])
                    nc.vector.bn_aggr(out=mv2[:gs, t, :], in_=st6b[:gs])
                lnv2 = st.tile([128, 3], f32, tag="lnv2" + L, name="lnv2")
                nc.scalar.activation(lnv2, mv2[:, :, 1], AF.Ln, bias=eps)
                nc.scalar.activation(rstd2, lnv2, AF.Exp, scale=-0.5)
                yield

                # ---- S15: h2 = (x2 - m2) * rstd2 -> next h_nat ----
                h_nat = st.tile([128, 3, E], f32r, tag="hnat" + L, name="h_nat")
                for t in range(3):
                    gs = GC[t][1] - GC[t][0]
                    nc.gpsimd.tensor_scalar(
                        out=h_nat[:gs, t, :], in0=x2[:gs, t, :],
                        scalar1=mv2[:gs, t, 0:1], scalar2=rstd2[:gs, t:t + 1],
                        op0=ALU.subtract, op1=ALU.mult)
                yield

                # ---- S16: hT = transpose(h2) ----
                hT = st.tile([128, 2, 304], f32r, tag="hT" + L, name="hT")
                for m in range(2):
                    tpo = ps.tile([128, VN], f32r, tag="ps", name=f"tpo{m}")
                    for t in range(3):
                        g0, g1 = GC2[t]
                        gs = g1 - g0
                        nc.tensor.transpose(tpo[:, g0:g1],
                                            h_nat[:gs, t, m * 128:(m + 1) * 128],
                                            ident_r[:gs, :gs])
                    ecopy(pick(), hT[:, m, :G], tpo[:, :G])
                yield

            if dbg and b == 0:
                nc.sync.dma_start(out=dbg_hl0[:], in_=h_nat.bitcast(f32))
            # ================= decoder =================
            ge_sb = st.tile([128, 2, 2], f32r, tag="ge" + L, name="ge_sb")
            for m in range(2):
                gep = ps.tile([128, 2], f32, tag="psS", name=f"gep{m}", bufs=2)
                for t in range(3):
                    g0, g1 = GC[t]
                    gs = g1 - g0
                    mm(gep, h_nat[:gs, t, m * 128:(m + 1) * 128],
                       mcol[:gs, t, :], t == 0, t == 2)
                nc.vector.tensor_copy(out=ge_sb[:, m, :], in_=gep)
            c_sb = st.tile([128, 2, 2], f32r, tag="c" + L, name="c_sb")
            for m in range(2):
                cp = ps.tile([128, 2], f32, tag="psS", name=f"cp{m}", bufs=2)
                for k in range(2):
                    mm(cp, cw["mt"][:, k, m * 128:(m + 1) * 128],
                       ge_sb[:, k, :], k == 0, k == 1)
                nc.vector.tensor_copy(out=c_sb[:, m, :], in_=cp)
            cc = ps.tile([LH, 2], f32, tag="psS", name="cc", bufs=2)
            for m in range(2):
                mm(cc, hT[:, m, IH:IH + LH], c_sb[:, m, :], m == 0, m == 1)
            nc.scalar.copy(out=compat_cols[:, b:b + 1], in_=cc[:, 0:1])
            if dbg and b == 0:
                nc.sync.dma_start(out=dbg_ge[:], in_=ge_sb.bitcast(f32))
                nc.sync.dma_start(out=dbg_c[:], in_=c_sb.bitcast(f32))
                ccc = st.tile([LH, 2], f32, tag="ccc" + L, name="ccc")
                nc.vector.tensor_copy(out=ccc, in_=cc)
                nc.sync.dma_start(out=dbg_cc[:], in_=ccc)
            yield

        # ---- drive the lanes (software pipeline) ----
        nb = nbp if nbp is not None else bpc
        nlanes = min(NWAY, nb)
        active = [elem(i, i) for i in range(nlanes)]
        nextb = nlanes
        while active:
            done = []
            for i, g in enumerate(active):
                try:
                    next(g)
                except StopIteration:
                    if nextb < nb:
                        active[i] = elem(nextb, i)
                        nextb += 1
                    else:
                        done.append(i)
            for i in reversed(done):
                active.pop(i)

        # ================= batched tail =================
        vl = const.tile([bpc, 1], f32, tag="vl")
        nc.vector.reduce_sum(vl, mask_bt, axis=AX.X)
        ivl = const.tile([bpc, 1], f32, tag="ivl")
        nc.vector.reciprocal(ivl, vl)

        ctp = ps.tile([128, LH], f32, tag="ps", name="ctp")
        nc.tensor.transpose(ctp[:bpc, :LH], compat_cols[:, :bpc], ident[:LH, :LH])
        compat_sb = const.tile([bpc, LH], f32, tag="compat_sb")
        nc.vector.tensor_copy(compat_sb, ctp[:bpc, :LH])
        th = const.tile([bpc, LH], f32, tag="th")
        nc.scalar.activation(th, compat_sb, AF.Tanh, scale=ivl)
        ex = const.tile([bpc, LH], f32, tag="ex")
        es = const.tile([bpc, 1], f32, tag="es")
        nc.scalar.activation(ex, th, AF.Exp, scale=CLIP, accum_out=es)
        er = const.tile([bpc, 1], f32, tag="er")
        nc.vector.reciprocal(er, es)
        pm = const.tile([bpc, LH], f32, tag="pm")
        nc.vector.tensor_scalar_mul(pm, in0=ex, scalar1=er)
        nc.vector.tensor_tensor(out=pm, in0=pm, in1=lv_bt, op=mybir.AluOpType.mult)
        nc.vector.tensor_scalar_add(pm, in0=pm, scalar1=1e-20)
        rs2 = const.tile([bpc, 1], f32, tag="rs2")
        nc.vector.reduce_sum(rs2, pm, axis=AX.X)
        rr2 = const.tile([bpc, 1], f32, tag="rr2")
        nc.vector.reciprocal(rr2, rs2)
        ob = const.tile([bpc, LH], f32, tag="ob")
        nc.vector.tensor_scalar_mul(ob, in0=pm, scalar1=rr2)
        nc.sync.dma_start(out=out_d[:], in_=ob)

    nc.finalize()
    return nc


# ----------------------------------------------------------------------------
# public entry point
# ----------------------------------------------------------------------------
def kernel(**inputs):
    observation = np.asarray(inputs["observation"], np.float32)
    w = _prep_weights(inputs)

    from concourse.bass_utils import run_bass_kernel_spmd

    nc = _build(BPC)
    in_maps = []
    for i in range(NCORES):
        m = dict(w)
        m.update(_prep_obs(observation[i * BPC:(i + 1) * BPC]))
        in_maps.append(m)
    res = run_bass_kernel_spmd(nc, in_maps, list(range(NCORES)))
    out = np.concatenate([res.results[i]["out"] for i in range(NCORES)], axis=0)
    return out.astype(np.float32)


# revision 17
# speedup vs baseline: 1.3481x; 1.3481x over previous
"""Trainium2 Bass kernel for nn_AttentionModel (graph attention encoder + decoder).

Contract: kernel(**inputs) takes FULL unsharded numpy inputs (as produced by
reference.setup_inputs()) and returns the FULL [256, 100] float32 output.
Internally shards the batch (256) across 8 NeuronCores (32 each, pure data
parallel; weights replicated) and runs a fused Bass/Tile kernel per core.

v4: folded weights (Wq@Wk^T, Wv@Wo), transposed-score attention (no score
transposes), Ln/Exp-based layernorm rsqrt pinned to one activation table,
host-pretransposed observation, matmul-based decoder, merged two-bank PSUM
tiles with single wide drains, and N-way software pipelining across batch
elements to keep all engines fed.

Self-contained: hardcodes all shapes; no sibling imports.
"""

import os
import sys

for _p in ("/opt/trn_rl_repo", "/opt/pypackages"):
    if _p not in sys.path:
        sys.path.append(_p)

import numpy as np
from contextlib import ExitStack

# --- static architecture constants ---
B, IH, IL, LH, E, FFH, NL = 256, 200, 6, 100, 256, 512, 2
G = IH + LH + 1  # 301
CLIP = 10.0
SCALE = 1.0 / 16.0  # 1/sqrt(E)
NCORES = 8
BPC = B // NCORES  # 32 batch elements per core

NWAY = int(os.environ.get("KNWAY", "5"))      # software pipeline lanes
STBUFS = int(os.environ.get("KSTBUFS", "1"))  # sbuf bufs per tag
PSBUFS = int(os.environ.get("KPSBUFS", "2"))  # merged (2-bank) psum bufs
PS1BUFS = int(os.environ.get("KPS1BUFS", "2"))  # single-bank psum bufs

GC = [(0, 128), (128, 256), (256, 301)]   # g chunks (written ranges)
GC2 = [(0, 128), (128, 256), (256, 302)]  # even-padded ranges for fp32r
VN = 302  # padded moving width over the node axis
SEGS = [(0, IH, "i"), (IH, IH + LH, "l"), (IH + LH, G, "n")]  # embed type slices


# ----------------------------------------------------------------------------
# host-side weight packing
# ----------------------------------------------------------------------------
def _tf32(x):
    """Round fp32 array to tfloat32 (10 mantissa bits), round-to-nearest-even."""
    u = np.ascontiguousarray(x, np.float32).view(np.uint32)
    u = (u + 0x0FFF + ((u >> 13) & 1)) & np.uint32(0xFFFFE000)
    return u.view(np.float32)


def _pack_rows(m, nchunk):
    """[nchunk*128, N] -> [128, nchunk, N] with [:, k, :] = m[128k:128(k+1), :]"""
    return np.ascontiguousarray(
        np.stack([m[i * 128:(i + 1) * 128] for i in range(nchunk)], axis=1)
    ).astype(np.float32)


def _prep_weights(inp):
    w = {}
    # --- embedding (fp16) ---
    for t, wk1, bk1, wk2, bk2 in (
        ("i", "wi1", "bi1", "wi2", "bi2"),
        ("l", "wl1", "bl1", "wl2", "bl2"),
        ("n", "wn1", "bn1", "wn2", "bn2"),
    ):
        w[f"w1_{t}"] = np.asarray(inp[wk1], np.float16)
        w[f"b1r_{t}"] = np.asarray(inp[bk1], np.float16).reshape(1, 32)
        w[f"w2_{t}"] = np.asarray(inp[wk2], np.float16)           # [32, 256]
        w[f"b2r_{t}"] = np.asarray(inp[bk2], np.float16).reshape(1, E)
    # --- encoder layers (f32r) ---
    for l in range(NL):
        wqk = (np.asarray(inp["enc_wq"][l], np.float64) * SCALE) @ np.asarray(
            inp["enc_wk"][l], np.float64).T
        wvo = np.asarray(inp["enc_wv"][l], np.float64) @ np.asarray(
            inp["enc_wo"][l], np.float64)
        w[f"wqk{l}"] = _tf32(_pack_rows(wqk.astype(np.float32), 2))   # lhsT chunks
        w[f"wvo{l}"] = _tf32(_pack_rows(wvo.astype(np.float32), 2))   # rhs chunks
        w[f"wf1{l}"] = _tf32(_pack_rows(inp["enc_wf1"][l], 2))        # [128,2,512] lhsT
        w[f"bf1r{l}"] = _tf32(np.asarray(inp["enc_bf1"][l], np.float32).reshape(1, FFH))
        w[f"wf2{l}"] = _tf32(_pack_rows(inp["enc_wf2"][l], 4))        # [128,4,256] rhs
        w[f"bf2r{l}"] = _tf32(np.asarray(inp["enc_bf2"][l], np.float32).reshape(1, E))
    # decoder fused matrix: compat = h_leaf . (M @ ge), M = Wpn_E @ Wfc.T
    MT = (np.asarray(inp["w_fc"], np.float64) @ np.asarray(
        inp["w_pn"], np.float64)[:, :E].T) * SCALE
    w["mt"] = _tf32(_pack_rows(MT.astype(np.float32), 2))             # [128,2,256] lhsT
    return w


def _prep_obs(observation):
    """Per-core observation-derived arrays (host-side layout transforms)."""
    obs = np.asarray(observation, np.float32)
    nb = obs.shape[0]
    # transposed features (values pre-rounded through fp16), g padded -> 304
    xt = np.zeros((nb, 9, 304), np.float32)
    xt[:, :, :G] = obs.transpose(0, 2, 1).astype(np.float16).astype(np.float32)
    # mask columns [nb, 128, 3, 2]: mcol[b, p, t, :] = mask[b, 128t + p]
    mask = obs[:, :, 8]
    mpad = np.zeros((nb, 384), np.float32)
    mpad[:, :G] = mask
    mcol = np.repeat(mpad.reshape(nb, 3, 128).transpose(0, 2, 1)[..., None], 2, axis=3)
    mcol = np.ascontiguousarray(mcol)
    # batched tail masks
    maskbt = np.zeros((nb, 304), np.float32)
    maskbt[:, :G] = mask
    lvbt = np.ascontiguousarray(mask[:, IH:IH + LH])
    return {"xt": xt, "mcol": mcol, "maskbt": maskbt, "lvbt": lvbt}


# ----------------------------------------------------------------------------
# numpy mirror of the device computation (for algebra validation)
# ----------------------------------------------------------------------------
def _numpy_mirror(observation, w):
    obs = np.asarray(observation, np.float32)
    nb = obs.shape[0]
    out = np.zeros((nb, LH), np.float32)

    def lrelu(x):
        return np.maximum(x, 0.01 * x)

    def ln(x):
        m = x.mean(-1, keepdims=True)
        v = x.var(-1, keepdims=True)
        return (x - m) * (v + 1e-5) ** -0.5

    for b in range(nb):
        xT = obs[b].T.astype(np.float16).astype(np.float32)  # [9, 301]
        h = np.zeros((G, E), np.float32)
        for (c0, c1, ty), nf in zip(SEGS, (6, 8, 6)):
            z = xT[:nf, c0:c1].T @ w[f"w1_{ty}"].astype(np.float32) \
                + w[f"b1r_{ty}"].astype(np.float32)
            h[c0:c1] = lrelu(z) @ w[f"w2_{ty}"].astype(np.float32) \
                + w[f"b2r_{ty}"].astype(np.float32)

        for l in range(NL):
            wqk = np.concatenate([w[f"wqk{l}"][:, 0], w[f"wqk{l}"][:, 1]], 0)
            wvo = np.concatenate([w[f"wvo{l}"][:, 0], w[f"wvo{l}"][:, 1]], 0)
            wf1 = np.concatenate([w[f"wf1{l}"][:, 0], w[f"wf1{l}"][:, 1]], 0)
            wf2 = np.concatenate([w[f"wf2{l}"][:, k] for k in range(4)], 0)
            u = h @ wqk
            sT = h @ u.T                      # sT[k, q]
            aT = np.exp(sT)
            rs = aT.sum(0)                    # per q
            o = (aT.T @ h) @ wvo              # [q, e]
            x1 = o / rs[:, None] + h
            h1 = ln(x1)
            p = np.maximum(h1 @ wf1 + w[f"bf1r{l}"], 0.0)
            x2 = p @ wf2 + w[f"bf2r{l}"] + h1
            h = ln(x2)

        mask = obs[b, :, 8]
        ge = (h * mask[:, None]).sum(0)       # unnormalized
        MT = np.concatenate([w["mt"][:, 0], w["mt"][:, 1]], 0)
        c = MT.T @ ge
        compat = h[IH:IH + LH] @ c            # unmasked leaf compat
        vlen = mask.sum()
        logits = np.tanh(compat / vlen) * CLIP
        ee = np.exp(logits)
        pp = ee / ee.sum()
        lv = mask[IH:IH + LH]
        masked = pp * lv + 1e-20
        out[b] = masked / masked.sum()
    return out


# ----------------------------------------------------------------------------
# the Bass/Tile kernel
# ----------------------------------------------------------------------------
def _build(bpc, nbp=None):
    import concourse.bass as bass
    import concourse.mybir as mybir
    import concourse.tile as tile
    from concourse import bacc
    from concourse.masks import make_identity

    f32 = mybir.dt.float32
    f32r = mybir.dt.float32r
    f16 = mybir.dt.float16
    AF = mybir.ActivationFunctionType
    ALU = mybir.AluOpType
    AX = mybir.AxisListType

    # Steer the act-table chooser: greedy first-match would pick tables that
    # split Ln and Exp, reloading on every layernorm. Present a view where the
    # shared funcs resolve only to natural_log_exp_and_others (indices are
    # preserved, so emitted act_func_set_ids stay valid for act_info.json).
    import concourse.hw_specs as _hw_specs
    _real_gat = _hw_specs.get_activation_tables

    def _patched_gat(arch):
        t = dict(_real_gat(arch))
        keep = "natural_log_exp_and_others"
        shared = {
            AF.Exp, AF.Ln, AF.Identity, AF.Copy, AF.Relu, AF.Prelu, AF.Square,
        }
        out = {}
        for name, funcs in t.items():
            out[name] = set(funcs) if name == keep else set(funcs) - shared
        return out

    bacc.get_activation_tables = _patched_gat

    nc = bacc.Bacc(None, target_bir_lowering=False)

    xt_d = nc.declare_dram_parameter("xt", [bpc, 9, 304], f32, isOutput=False)
    mcol_d = nc.declare_dram_parameter("mcol", [bpc, 128, 3, 2], f32r, isOutput=False)
    maskbt_d = nc.declare_dram_parameter("maskbt", [bpc, 304], f32, isOutput=False)
    lvbt_d = nc.declare_dram_parameter("lvbt", [bpc, LH], f32, isOutput=False)
    dp = {}
    for t in "iln":
        nf = 8 if t == "l" else 6
        dp[f"w1_{t}"] = nc.declare_dram_parameter(f"w1_{t}", [nf, 32], f16, isOutput=False)
        dp[f"b1r_{t}"] = nc.declare_dram_parameter(f"b1r_{t}", [1, 32], f16, isOutput=False)
        dp[f"w2_{t}"] = nc.declare_dram_parameter(f"w2_{t}", [32, E], f16, isOutput=False)
        dp[f"b2r_{t}"] = nc.declare_dram_parameter(f"b2r_{t}", [1, E], f16, isOutput=False)
    for l in range(NL):
        dp[f"wqk{l}"] = nc.declare_dram_parameter(f"wqk{l}", [128, 2, E], f32r, isOutput=False)
        dp[f"wvo{l}"] = nc.declare_dram_parameter(f"wvo{l}", [128, 2, E], f32r, isOutput=False)
        dp[f"wf1{l}"] = nc.declare_dram_parameter(f"wf1{l}", [128, 2, FFH], f32r, isOutput=False)
        dp[f"bf1r{l}"] = nc.declare_dram_parameter(f"bf1r{l}", [1, FFH], f32r, isOutput=False)
        dp[f"wf2{l}"] = nc.declare_dram_parameter(f"wf2{l}", [128, 4, E], f32r, isOutput=False)
        dp[f"bf2r{l}"] = nc.declare_dram_parameter(f"bf2r{l}", [1, E], f32r, isOutput=False)
    dp["mt"] = nc.declare_dram_parameter("mt", [128, 2, E], f32r, isOutput=False)
    out_d = nc.declare_dram_parameter("out", [bpc, LH], f32, isOutput=True)

    with tile.TileContext(nc) as tc, ExitStack() as ctx:
        const = ctx.enter_context(tc.tile_pool(name="const", bufs=1))
        st = ctx.enter_context(tc.tile_pool(name="st", bufs=STBUFS))
        ps = ctx.enter_context(tc.tile_pool(name="ps", bufs=PSBUFS, space="PSUM"))

        # ---- constants / weights into SBUF ----
        ident = const.tile([128, 128], f32, tag="ident")
        make_identity(nc, ident)
        ident_r = const.tile([128, 128], f32r, tag="ident_r")
        nc.vector.tensor_copy(out=ident_r, in_=ident)
        ones_f = const.tile([1, 512], f32, tag="ones_f")
        nc.vector.memset(ones_f, 1.0)
        ones_r = const.tile([1, 512], f32r, tag="ones_r")
        nc.vector.tensor_copy(out=ones_r, in_=ones_f)
        ones16 = const.tile([1, 512], f16, tag="ones16")
        nc.vector.tensor_copy(out=ones16, in_=ones_f)
        eps = const.tile([128, 1], f32, tag="eps")
        nc.vector.memset(eps, 1e-5)
        onecol_f = const.tile([128, 2], f32, tag="onecol_f")
        nc.vector.memset(onecol_f, 1.0)
        onecol2_r = const.tile([128, 2], f32r, tag="onecol2_r")
        nc.vector.tensor_copy(out=onecol2_r, in_=onecol_f)

        cw = {}
        for nm, h in dp.items():
            t = const.tile(list(h.shape), h.dtype, tag=f"w_{nm}")
            nc.sync.dma_start(out=t, in_=h[:])
            cw[nm] = t

        mask_bt = const.tile([bpc, 304], f32, tag="mask_bt")
        nc.sync.dma_start(out=mask_bt, in_=maskbt_d[:])
        lv_bt = const.tile([bpc, LH], f32, tag="lv_bt")
        nc.sync.dma_start(out=lv_bt, in_=lvbt_d[:])

        compat_cols = const.tile([LH, max(bpc, 2)], f32, tag="compat_cols")

        def mm(out, lhsT, rhs, start, stop):
            nc.tensor.matmul(out, lhsT, rhs, start=start, stop=stop)

        ENGS2 = (nc.scalar, nc.vector)

        def ecopy(eng, out, in_):
            if eng is nc.scalar:
                nc.scalar.copy(out=out, in_=in_)
            else:
                eng.tensor_copy(out=out, in_=in_)

        def ps2(name, dt=f32):
            """Two-bank merged psum tile; each 512-f32 region holds one matmul."""
            return ps.tile([128, 2, 512], dt, tag="ps2", name=name, bufs=PSBUFS)

        def ps1(name, dt=f32):
            return ps.tile([128, 512], dt, tag="ps1", name=name, bufs=PS1BUFS)

        # ================= per batch element (generator, staged) =============
        def elem(b, lane):
            L = str(lane)
            rr = [lane]  # rotating engine picker for copies

            def pick():
                e = ENGS2[rr[0] % 2]
                rr[0] += 1
                return e

            # ---- S0: loads ----
            xT32 = st.tile([9, 304], f32, tag="xT32" + L, name="xT32")
            nc.sync.dma_start(out=xT32, in_=xt_d[b])
            xT = st.tile([9, 304], f16, tag="xT" + L, name="xT")
            nc.vector.tensor_copy(out=xT, in_=xT32)
            mcol = st.tile([128, 3, 2], f32r, tag="mcol" + L, name="mcol")
            nc.sync.dma_start(out=mcol, in_=mcol_d[b])
            yield

            # ---- S1: z1 = W1^T x + b1 ; lr = leakyrelu(z1) ----
            z1 = ps1("z1")
            for (c0, c1, ty), nf in zip(SEGS, (6, 8, 6)):
                mm(z1[:32, c0:c1], cw[f"w1_{ty}"], xT[:nf, c0:c1], True, False)
                mm(z1[:32, c0:c1], cw[f"b1r_{ty}"], ones16[:, :c1 - c0], False, True)
            small1 = st.tile([32, 304], f16, tag="small1" + L, name="small1")
            nc.scalar.activation(small1[:, :G], z1[:32, :G], AF.Identity, scale=0.01)
            lr = st.tile([32, 304], f16, tag="lr" + L, name="lr")
            nc.vector.tensor_tensor(out=lr[:, :G], in0=z1[:32, :G], in1=small1[:, :G],
                                    op=ALU.max)
            yield

            # ---- S2: h0T[e, g] = W2^T lr + b2 (direct transposed embed) ----
            hT = st.tile([128, 2, 304], f32r, tag="hT" + L, name="hT")
            hp = ps2("h0p")
            for m in range(2):
                for si, (c0, c1, ty) in enumerate(SEGS):
                    mm(hp[:, m, c0:c1], cw[f"w2_{ty}"][:, m * 128:(m + 1) * 128],
                       lr[:, c0:c1], True, False)
                    mm(hp[:, m, c0:c1], cw[f"b2r_{ty}"][:, m * 128:(m + 1) * 128],
                       ones16[:, :c1 - c0], False, True)
            ecopy(pick(), hT[:, :, :G], hp[:, :, :G])
            yield

            # ---- S3: h_nat = transpose(hT) ----
            h_nat = st.tile([128, 3, E], f32r, tag="hnat" + L, name="h_nat")
            tpn = ps2("tpn", f32r)
            for t in range(2):
                g0, g1 = GC2[t]
                for m in range(2):
                    nc.tensor.transpose(tpn[:g1 - g0, t, m * 128:(m + 1) * 128],
                                        hT[:, m, g0:g1], ident_r)
            ecopy(pick(), h_nat[:, 0:2, :], tpn[:, :, :E])
            g0, g1 = GC2[2]
            tpn2 = ps1("tpn2", f32r)
            for m in range(2):
                nc.tensor.transpose(tpn2[:g1 - g0, m * 128:(m + 1) * 128],
                                    hT[:, m, g0:g1], ident_r)
            ecopy(pick(), h_nat[:GC[2][1] - g0, 2, :], tpn2[:GC[2][1] - g0, :E])
            yield

            # ================= encoder layers =================
            for l in range(NL):
                # ---- S4: uT[e', g] = WQK^T hT ----
                u = st.tile([128, 2, 304], f32r, tag="uv" + L, name="u")
                up = ps2("up")
                for m in range(2):
                    for k in range(2):
                        mm(up[:, m, :VN], cw[f"wqk{l}"][:, k, m * 128:(m + 1) * 128],
                           hT[:, k, :VN], k == 0, k == 1)
                ecopy(pick(), u[:, :, :VN], up[:, :, :VN])
                yield

                # ---- S5: sT[k, q] = hT^T u ; aT = exp(sT) ----
                aT = st.tile([128, 4, 304], f32r, tag="ap" + L, name="aT")
                sp = ps2("sp")
                for t in range(2):
                    g0, g1 = GC[t]
                    for m in range(2):
                        mm(sp[:, t, :VN], hT[:, m, g0:g1], u[:, m, :VN],
                           m == 0, m == 1)
                nc.scalar.activation(aT[:, 0:2, :VN], sp[:, :, :VN], AF.Exp)
                g0, g1 = GC[2]
                sp2 = ps1("sp2")
                for m in range(2):
                    mm(sp2[:g1 - g0, :VN], hT[:, m, g0:g1], u[:, m, :VN],
                       m == 0, m == 1)
                nc.scalar.activation(aT[:g1 - g0, 2, :VN], sp2[:g1 - g0, :VN], AF.Exp)
                yield

                # ---- S6: rsum per q (column form) ; rinv = 1/rsum ----
                rs = ps.tile([128, 6], f32, tag="psS", name="rs", bufs=2)
                for tq in range(3):
                    q0, q1 = GC2[tq]
                    for tk in range(3):
                        k0, k1 = GC[tk]
                        ks = k1 - k0
                        mm(rs[:q1 - q0, 2 * tq:2 * tq + 2], aT[:ks, tk, q0:q1],
                           onecol2_r[:ks], tk == 0, tk == 2)
                rinv = st.tile([128, 3], f32, tag="rinv" + L, name="rinv")
                for tq in range(3):
                    q0, q1 = GC2[tq]
                    nc.vector.reciprocal(rinv[:q1 - q0, tq:tq + 1],
                                         rs[:q1 - q0, 2 * tq:2 * tq + 1])

                # ---- S7: avT[e, q] = h_nat^T aT ----
                avT = st.tile([128, 2, 304], f32r, tag="uv" + L, name="avT")
                ap_ = ps2("avp")
                for m in range(2):
                    for tk in range(3):
                        k0, k1 = GC[tk]
                        ks = k1 - k0
                        mm(ap_[:, m, :VN], h_nat[:ks, tk, m * 128:(m + 1) * 128],
                           aT[:ks, tk, :VN], tk == 0, tk == 2)
                ecopy(pick(), avT[:, :, :VN], ap_[:, :, :VN])
                yield

                # ---- S8: o = av @ WVO ; x1 = o*rinv + h (fused) ----
                x1 = st.tile([128, 3, E], f32, tag="x" + L, name="x1")
                op2 = ps2("op2")
                for t in range(2):
                    g0, g1 = GC[t]
                    for k in range(2):
                        mm(op2[:, t, :E], avT[:, k, g0:g1], cw[f"wvo{l}"][:, k, :],
                           k == 0, k == 1)
                op1 = ps1("op1")
                g0, g1 = GC[2]
                for k in range(2):
                    mm(op1[:g1 - g0, :E], avT[:, k, g0:g1], cw[f"wvo{l}"][:, k, :],
                       k == 0, k == 1)
                for t in range(2):
                    nc.vector.scalar_tensor_tensor(
                        out=x1[:, t, :], in0=op2[:, t, :E],
                        scalar=rinv[:, t:t + 1], in1=h_nat[:, t, :],
                        op0=ALU.mult, op1=ALU.add)
                gs = G - GC[2][0]
                nc.vector.scalar_tensor_tensor(
                    out=x1[:gs, 2, :], in0=op1[:gs, :E],
                    scalar=rinv[:gs, 2:3], in1=h_nat[:gs, 2, :],
                    op0=ALU.mult, op1=ALU.add)
                yield

                # ---- S9: LN1 stats ----
                mv1 = st.tile([128, 3, 2], f32, tag="mv1" + L, name="mv1")
                st6 = st.tile([128, 3, 6], f32, tag="st6" + L, name="st6", bufs=2)
                nc.vector.bn_stats(out=st6[:, 0:2, :], in_=x1[:, 0:2, :])
                nc.vector.bn_aggr(out=mv1[:, 0, :], in_=st6[:, 0, :])
                nc.vector.bn_aggr(out=mv1[:, 1, :], in_=st6[:, 1, :])
                gs = G - GC[2][0]
                nc.vector.bn_stats(out=st6[:gs, 2, :], in_=x1[:gs, 2, :])
                nc.vector.bn_aggr(out=mv1[:gs, 2, :], in_=st6[:gs, 2, :])
                lnv1 = st.tile([128, 3], f32, tag="lnv1" + L, name="lnv1")
                nc.scalar.activation(lnv1, mv1[:, :, 1], AF.Ln, bias=eps)
                rstd1 = st.tile([128, 3], f32, tag="rstd1" + L, name="rstd1")
                nc.scalar.activation(rstd1, lnv1, AF.Exp, scale=-0.5)
                yield

                # ---- S10: h1 = (x1 - m1) * rstd1 ----
                h1 = st.tile([128, 3, E], f32r, tag="h1" + L, name="h1")
                for t in range(3):
                    gs = GC[t][1] - GC[t][0]
                    nc.gpsimd.tensor_scalar(
                        out=h1[:gs, t, :], in0=x1[:gs, t, :],
                        scalar1=mv1[:gs, t, 0:1], scalar2=rstd1[:gs, t:t + 1],
                        op0=ALU.subtract, op1=ALU.mult)
                yield

                # ---- S11: h1T = transpose(h1) ----
                h1T = st.tile([128, 2, 304], f32r, tag="h1T" + L, name="h1T")
                tph = ps2("tph", f32r)
                for m in range(2):
                    for t in range(3):
                        g0, g1 = GC2[t]
                        gs = g1 - g0
                        nc.tensor.transpose(tph[:, m, g0:g1],
                                            h1[:gs, t, m * 128:(m + 1) * 128],
                                            ident_r[:gs, :gs])
                ecopy(pick(), h1T[:, :, :G], tph[:, :, :G])
                yield

                # ---- S12: f1 = relu(Wf1^T h1T + bf1) ----
                p = st.tile([128, 4, 304], f32r, tag="ap" + L, name="p")
                for half in range(2):
                    fp = ps2("fp")
                    for j in range(2):
                        m = half * 2 + j
                        for k in range(2):
                            mm(fp[:, j, :VN],
                               cw[f"wf1{l}"][:, k, m * 128:(m + 1) * 128],
                               h1T[:, k, :VN], k == 0, False)
                        mm(fp[:, j, :VN], cw[f"bf1r{l}"][:, m * 128:(m + 1) * 128],
                           ones_r[:, :VN], False, True)
                    nc.scalar.activation(p[:, 2 * half:2 * half + 2, :G],
                                         fp[:, :, :G], AF.Relu)
                yield

                # ---- S13: f2 + bf2 ; x2 = f2 + h1 (fused add) ----
                x2 = st.tile([128, 3, E], f32, tag="x" + L, name="x2")
                f2p = ps2("f2p")
                for t in range(2):
                    g0, g1 = GC[t]
                    for m in range(4):
                        mm(f2p[:, t, :E], p[:, m, g0:g1], cw[f"wf2{l}"][:, m, :],
                           m == 0, False)
                    mm(f2p[:, t, :E], ones_r[:, :g1 - g0], cw[f"bf2r{l}"],
                       False, True)
                nc.vector.tensor_tensor(out=x2[:, 0:2, :], in0=f2p[:, :, :E],
                                        in1=h1[:, 0:2, :], op=ALU.add)
                f2p1 = ps1("f2p1")
                g0, g1 = GC[2]
                gs = g1 - g0
                for m in range(4):
                    mm(f2p1[:gs, :E], p[:, m, g0:g1], cw[f"wf2{l}"][:, m, :],
                       m == 0, False)
                mm(f2p1[:gs, :E], ones_r[:, :gs], cw[f"bf2r{l}"], False, True)
                nc.vector.tensor_tensor(out=x2[:gs, 2, :], in0=f2p1[:gs, :E],
                                        in1=h1[:gs, 2, :], op=ALU.add)
                yield

                # ---- S14: LN2 stats ----
                mv2 = st.tile([128, 3, 2], f32, tag="mv2" + L, name="mv2")
                st6b = st.tile([128, 3, 6], f32, tag="st6" + L, name="st6b", bufs=2)
                nc.vector.bn_stats(out=st6b[:, 0:2, :], in_=x2[:, 0:2, :])
                nc.vector.bn_aggr(out=mv2[:, 0, :], in_=st6b[:, 0, :])
                nc.vector.bn_aggr(out=mv2[:, 1, :], in_=st6b[:, 1, :])
                gs = G - GC[2][0]
                nc.vector.bn_stats(out=st6b[:gs, 2, :], in_=x2[:gs, 2, :])
                nc.vector.bn_aggr(out=mv2[:gs, 2, :], in_=st6b[:gs, 2, :])
                lnv2 = st.tile([128, 3], f32, tag="lnv2" + L, name="lnv2")
                nc.scalar.activation(lnv2, mv2[:, :, 1], AF.Ln, bias=eps)
                rstd2 = st.tile([128, 3], f32, tag="rstd2" + L, name="rstd2")
                nc.scalar.activation(rstd2, lnv2, AF.Exp, scale=-0.5)
                yield

                # ---- S15: h2 = (x2 - m2) * rstd2 -> next h_nat ----
                h_nat = st.tile([128, 3, E], f32r, tag="hnat" + L, name="h_nat")
                for t in range(3):
                    gs = GC[t][1] - GC[t][0]
                    nc.gpsimd.tensor_scalar(
                        out=h_nat[:gs, t, :], in0=x2[:gs, t, :],
                        scalar1=mv2[:gs, t, 0:1], scalar2=rstd2[:gs, t:t + 1],
                        op0=ALU.subtract, op1=ALU.mult)
                yield

                # ---- S16: hT = transpose(h2) ----
                hT = st.tile([128, 2, 304], f32r, tag="hT" + L, name="hT")
                tpo = ps2("tpo", f32r)
                for m in range(2):
                    for t in range(3):
                        g0, g1 = GC2[t]
                        gs = g1 - g0
                        nc.tensor.transpose(tpo[:, m, g0:g1],
                                            h_nat[:gs, t, m * 128:(m + 1) * 128],
                                            ident_r[:gs, :gs])
                ecopy(pick(), hT[:, :, :G], tpo[:, :, :G])
                yield

            # ================= decoder =================
            ge_sb = st.tile([128, 2, 2], f32r, tag="ge" + L, name="ge_sb")
            for m in range(2):
                gep = ps.tile([128, 2], f32, tag="psS", name=f"gep{m}", bufs=2)
                for t in range(3):
                    g0, g1 = GC[t]
                    gs = g1 - g0
                    mm(gep, h_nat[:gs, t, m * 128:(m + 1) * 128],
                       mcol[:gs, t, :], t == 0, t == 2)
                nc.vector.tensor_copy(out=ge_sb[:, m, :], in_=gep)
            c_sb = st.tile([128, 2, 2], f32r, tag="c" + L, name="c_sb")
            for m in range(2):
                cp = ps.tile([128, 2], f32, tag="psS", name=f"cp{m}", bufs=2)
                for k in range(2):
                    mm(cp, cw["mt"][:, k, m * 128:(m + 1) * 128],
                       ge_sb[:, k, :], k == 0, k == 1)
                nc.vector.tensor_copy(out=c_sb[:, m, :], in_=cp)
            cc = ps.tile([LH, 2], f32, tag="psS", name="cc", bufs=2)
            for m in range(2):
                mm(cc, hT[:, m, IH:IH + LH], c_sb[:, m, :], m == 0, m == 1)
            nc.scalar.copy(out=compat_cols[:, b:b + 1], in_=cc[:, 0:1])
            yield

        # ---- drive the lanes (software pipeline) ----
        nb = nbp if nbp is not None else bpc
        nlanes = min(NWAY, nb)
        active = [elem(i, i) for i in range(nlanes)]
        nextb = nlanes
        while active:
            done = []
            for i, g in enumerate(active):
                try:
                    next(g)
                except StopIteration:
                    if nextb < nb:
                        active[i] = elem(nextb, i)
                        nextb += 1
                    else:
                        done.append(i)
            for i in reversed(done):
                active.pop(i)

        # ================= batched tail =================
        vl = const.tile([bpc, 1], f32, tag="vl")
        nc.vector.reduce_sum(vl, mask_bt, axis=AX.X)
        ivl = const.tile([bpc, 1], f32, tag="ivl")
        nc.vector.reciprocal(ivl, vl)

        ctp = ps.tile([128, LH], f32, tag="psS", name="ctp", bufs=2)
        nc.tensor.transpose(ctp[:bpc, :LH], compat_cols[:, :bpc], ident[:LH, :LH])
        compat_sb = const.tile([bpc, LH], f32, tag="compat_sb")
        nc.vector.tensor_copy(compat_sb, ctp[:bpc, :LH])
        th = const.tile([bpc, LH], f32, tag="th")
        nc.scalar.activation(th, compat_sb, AF.Tanh, scale=ivl)
        ex = const.tile([bpc, LH], f32, tag="ex")
        es = const.tile([bpc, 1], f32, tag="es")
        nc.scalar.activation(ex, th, AF.Exp, scale=CLIP, accum_out=es)
        er = const.tile([bpc, 1], f32, tag="er")
        nc.vector.reciprocal(er, es)
        pm = const.tile([bpc, LH], f32, tag="pm")
        nc.vector.tensor_scalar_mul(pm, in0=ex, scalar1=er)
        nc.vector.tensor_tensor(out=pm, in0=pm, in1=lv_bt, op=mybir.AluOpType.mult)
        nc.vector.tensor_scalar_add(pm, in0=pm, scalar1=1e-20)
        rs2 = const.tile([bpc, 1], f32, tag="rs2")
        nc.vector.reduce_sum(rs2, pm, axis=AX.X)
        rr2 = const.tile([bpc, 1], f32, tag="rr2")
        nc.vector.reciprocal(rr2, rs2)
        ob = const.tile([bpc, LH], f32, tag="ob")
        nc.vector.tensor_scalar_mul(ob, in0=pm, scalar1=rr2)
        nc.sync.dma_start(out=out_d[:], in_=ob)

    nc.finalize()
    return nc


# ----------------------------------------------------------------------------
# public entry point
# ----------------------------------------------------------------------------
def kernel(**inputs):
    observation = np.asarray(inputs["observation"], np.float32)
    w = _prep_weights(inputs)

    from concourse.bass_utils import run_bass_kernel_spmd

    nc = _build(BPC)
    in_maps = []
    for i in range(NCORES):
        m = dict(w)
        m.update(_prep_obs(observation[i * BPC:(i + 1) * BPC]))
        in_maps.append(m)
    res = run_bass_kernel_spmd(nc, in_maps, list(range(NCORES)))
    out = np.concatenate([res.results[i]["out"] for i in range(NCORES)], axis=0)
    return out.astype(np.float32)


# revision 18
# speedup vs baseline: 1.5727x; 1.1666x over previous
"""Trainium2 Bass kernel for nn_AttentionModel (graph attention encoder + decoder).

Contract: kernel(**inputs) takes FULL unsharded numpy inputs (as produced by
reference.setup_inputs()) and returns the FULL [256, 100] float32 output.
Internally shards the batch (256) across 8 NeuronCores (32 each, pure data
parallel; weights replicated) and runs a fused Bass/Tile kernel per core.

v4: folded weights (Wq@Wk^T, Wv@Wo), transposed-score attention (no score
transposes), Ln/Exp-based layernorm rsqrt pinned to one activation table,
host-pretransposed observation, matmul-based decoder, merged two-bank PSUM
tiles with single wide drains, and N-way software pipelining across batch
elements to keep all engines fed.

Self-contained: hardcodes all shapes; no sibling imports.
"""

import os
import sys

for _p in ("/opt/trn_rl_repo", "/opt/pypackages"):
    if _p not in sys.path:
        sys.path.append(_p)

import numpy as np
from contextlib import ExitStack

# --- static architecture constants ---
B, IH, IL, LH, E, FFH, NL = 256, 200, 6, 100, 256, 512, 2
G = IH + LH + 1  # 301
CLIP = 10.0
SCALE = 1.0 / 16.0  # 1/sqrt(E)
NCORES = 8
BPC = B // NCORES  # 32 batch elements per core

NWAY = int(os.environ.get("KNWAY", "5"))      # software pipeline lanes
STBUFS = int(os.environ.get("KSTBUFS", "1"))  # sbuf bufs per tag
PSBUFS = int(os.environ.get("KPSBUFS", "2"))  # merged (2-bank) psum bufs
PS1BUFS = int(os.environ.get("KPS1BUFS", "2"))  # single-bank psum bufs
SKEW = int(os.environ.get("KSKEW", "0"))      # lane start stagger (rounds)

GC = [(0, 128), (128, 256), (256, 301)]   # g chunks (written ranges)
GC2 = [(0, 128), (128, 256), (256, 302)]  # even-padded ranges for fp32r
VN = 302  # padded moving width over the node axis
SEGS = [(0, IH, "i"), (IH, IH + LH, "l"), (IH + LH, G, "n")]  # embed type slices


# ----------------------------------------------------------------------------
# host-side weight packing
# ----------------------------------------------------------------------------
def _tf32(x):
    """Round fp32 array to tfloat32 (10 mantissa bits), round-to-nearest-even."""
    u = np.ascontiguousarray(x, np.float32).view(np.uint32)
    u = (u + 0x0FFF + ((u >> 13) & 1)) & np.uint32(0xFFFFE000)
    return u.view(np.float32)


def _pack_rows(m, nchunk):
    """[nchunk*128, N] -> [128, nchunk, N] with [:, k, :] = m[128k:128(k+1), :]"""
    return np.ascontiguousarray(
        np.stack([m[i * 128:(i + 1) * 128] for i in range(nchunk)], axis=1)
    ).astype(np.float32)


def _prep_weights(inp):
    w = {}
    # --- embedding (fp16) ---
    for t, wk1, bk1, wk2, bk2 in (
        ("i", "wi1", "bi1", "wi2", "bi2"),
        ("l", "wl1", "bl1", "wl2", "bl2"),
        ("n", "wn1", "bn1", "wn2", "bn2"),
    ):
        w[f"w1_{t}"] = np.asarray(inp[wk1], np.float16)
        w[f"b1r_{t}"] = np.asarray(inp[bk1], np.float16).reshape(1, 32)
        w[f"w2_{t}"] = np.asarray(inp[wk2], np.float16)           # [32, 256]
        w[f"b2r_{t}"] = np.asarray(inp[bk2], np.float16).reshape(1, E)
    # --- encoder layers (f32r) ---
    for l in range(NL):
        wqk = (np.asarray(inp["enc_wq"][l], np.float64) * SCALE) @ np.asarray(
            inp["enc_wk"][l], np.float64).T
        wvo = np.asarray(inp["enc_wv"][l], np.float64) @ np.asarray(
            inp["enc_wo"][l], np.float64)
        w[f"wqk{l}"] = _tf32(_pack_rows(wqk.astype(np.float32), 2))   # lhsT chunks
        w[f"wvo{l}"] = _tf32(_pack_rows(wvo.astype(np.float32), 2))   # rhs chunks
        w[f"wf1{l}"] = _tf32(_pack_rows(inp["enc_wf1"][l], 2))        # [128,2,512] lhsT
        w[f"bf1r{l}"] = _tf32(np.asarray(inp["enc_bf1"][l], np.float32).reshape(1, FFH))
        w[f"wf2{l}"] = _tf32(_pack_rows(inp["enc_wf2"][l], 4))        # [128,4,256] rhs
        w[f"bf2r{l}"] = _tf32(np.asarray(inp["enc_bf2"][l], np.float32).reshape(1, E))
    # decoder fused matrix: compat = h_leaf . (M @ ge), M = Wpn_E @ Wfc.T
    MT = (np.asarray(inp["w_fc"], np.float64) @ np.asarray(
        inp["w_pn"], np.float64)[:, :E].T) * SCALE
    w["mt"] = _tf32(_pack_rows(MT.astype(np.float32), 2))             # [128,2,256] lhsT
    return w


def _prep_obs(observation):
    """Per-core observation-derived arrays (host-side layout transforms)."""
    obs = np.asarray(observation, np.float32)
    nb = obs.shape[0]
    # transposed features (values pre-rounded through fp16), g padded -> 304
    xt = np.zeros((nb, 9, 304), np.float32)
    xt[:, :, :G] = obs.transpose(0, 2, 1).astype(np.float16).astype(np.float32)
    # mask columns [nb, 128, 3, 2]: mcol[b, p, t, :] = mask[b, 128t + p]
    mask = obs[:, :, 8]
    mpad = np.zeros((nb, 384), np.float32)
    mpad[:, :G] = mask
    mcol = np.repeat(mpad.reshape(nb, 3, 128).transpose(0, 2, 1)[..., None], 2, axis=3)
    mcol = np.ascontiguousarray(mcol)
    # batched tail masks
    maskbt = np.zeros((nb, 304), np.float32)
    maskbt[:, :G] = mask
    lvbt = np.ascontiguousarray(mask[:, IH:IH + LH])
    return {"xt": xt, "mcol": mcol, "maskbt": maskbt, "lvbt": lvbt}


# ----------------------------------------------------------------------------
# numpy mirror of the device computation (for algebra validation)
# ----------------------------------------------------------------------------
def _numpy_mirror(observation, w):
    obs = np.asarray(observation, np.float32)
    nb = obs.shape[0]
    out = np.zeros((nb, LH), np.float32)

    def lrelu(x):
        return np.maximum(x, 0.01 * x)

    def ln(x):
        m = x.mean(-1, keepdims=True)
        v = x.var(-1, keepdims=True)
        return (x - m) * (v + 1e-5) ** -0.5

    for b in range(nb):
        xT = obs[b].T.astype(np.float16).astype(np.float32)  # [9, 301]
        h = np.zeros((G, E), np.float32)
        for (c0, c1, ty), nf in zip(SEGS, (6, 8, 6)):
            z = xT[:nf, c0:c1].T @ w[f"w1_{ty}"].astype(np.float32) \
                + w[f"b1r_{ty}"].astype(np.float32)
            h[c0:c1] = lrelu(z) @ w[f"w2_{ty}"].astype(np.float32) \
                + w[f"b2r_{ty}"].astype(np.float32)

        for l in range(NL):
            wqk = np.concatenate([w[f"wqk{l}"][:, 0], w[f"wqk{l}"][:, 1]], 0)
            wvo = np.concatenate([w[f"wvo{l}"][:, 0], w[f"wvo{l}"][:, 1]], 0)
            wf1 = np.concatenate([w[f"wf1{l}"][:, 0], w[f"wf1{l}"][:, 1]], 0)
            wf2 = np.concatenate([w[f"wf2{l}"][:, k] for k in range(4)], 0)
            u = h @ wqk
            sT = h @ u.T                      # sT[k, q]
            aT = np.exp(sT)
            rs = aT.sum(0)                    # per q
            o = (aT.T @ h) @ wvo              # [q, e]
            x1 = o / rs[:, None] + h
            h1 = ln(x1)
            p = np.maximum(h1 @ wf1 + w[f"bf1r{l}"], 0.0)
            x2 = p @ wf2 + w[f"bf2r{l}"] + h1
            h = ln(x2)

        mask = obs[b, :, 8]
        ge = (h * mask[:, None]).sum(0)       # unnormalized
        MT = np.concatenate([w["mt"][:, 0], w["mt"][:, 1]], 0)
        c = MT.T @ ge
        compat = h[IH:IH + LH] @ c            # unmasked leaf compat
        vlen = mask.sum()
        logits = np.tanh(compat / vlen) * CLIP
        ee = np.exp(logits)
        pp = ee / ee.sum()
        lv = mask[IH:IH + LH]
        masked = pp * lv + 1e-20
        out[b] = masked / masked.sum()
    return out


# ----------------------------------------------------------------------------
# the Bass/Tile kernel
# ----------------------------------------------------------------------------
def _build(bpc, nbp=None):
    import concourse.bass as bass
    import concourse.mybir as mybir
    import concourse.tile as tile
    from concourse import bacc
    from concourse.masks import make_identity

    f32 = mybir.dt.float32
    f32r = mybir.dt.float32r
    f16 = mybir.dt.float16
    AF = mybir.ActivationFunctionType
    ALU = mybir.AluOpType
    AX = mybir.AxisListType

    # Steer the act-table chooser: greedy first-match would pick tables that
    # split Ln and Exp, reloading on every layernorm. Present a view where the
    # shared funcs resolve only to natural_log_exp_and_others (indices are
    # preserved, so emitted act_func_set_ids stay valid for act_info.json).
    import concourse.hw_specs as _hw_specs
    _real_gat = _hw_specs.get_activation_tables

    def _patched_gat(arch):
        t = dict(_real_gat(arch))
        keep = "natural_log_exp_and_others"
        shared = {
            AF.Exp, AF.Ln, AF.Identity, AF.Copy, AF.Relu, AF.Prelu, AF.Square,
        }
        out = {}
        for name, funcs in t.items():
            out[name] = set(funcs) if name == keep else set(funcs) - shared
        return out

    bacc.get_activation_tables = _patched_gat

    nc = bacc.Bacc(None, target_bir_lowering=False)

    xt_d = nc.declare_dram_parameter("xt", [bpc, 9, 304], f32, isOutput=False)
    mcol_d = nc.declare_dram_parameter("mcol", [bpc, 128, 3, 2], f32r, isOutput=False)
    maskbt_d = nc.declare_dram_parameter("maskbt", [bpc, 304], f32, isOutput=False)
    lvbt_d = nc.declare_dram_parameter("lvbt", [bpc, LH], f32, isOutput=False)
    dp = {}
    for t in "iln":
        nf = 8 if t == "l" else 6
        dp[f"w1_{t}"] = nc.declare_dram_parameter(f"w1_{t}", [nf, 32], f16, isOutput=False)
        dp[f"b1r_{t}"] = nc.declare_dram_parameter(f"b1r_{t}", [1, 32], f16, isOutput=False)
        dp[f"w2_{t}"] = nc.declare_dram_parameter(f"w2_{t}", [32, E], f16, isOutput=False)
        dp[f"b2r_{t}"] = nc.declare_dram_parameter(f"b2r_{t}", [1, E], f16, isOutput=False)
    for l in range(NL):
        dp[f"wqk{l}"] = nc.declare_dram_parameter(f"wqk{l}", [128, 2, E], f32r, isOutput=False)
        dp[f"wvo{l}"] = nc.declare_dram_parameter(f"wvo{l}", [128, 2, E], f32r, isOutput=False)
        dp[f"wf1{l}"] = nc.declare_dram_parameter(f"wf1{l}", [128, 2, FFH], f32r, isOutput=False)
        dp[f"bf1r{l}"] = nc.declare_dram_parameter(f"bf1r{l}", [1, FFH], f32r, isOutput=False)
        dp[f"wf2{l}"] = nc.declare_dram_parameter(f"wf2{l}", [128, 4, E], f32r, isOutput=False)
        dp[f"bf2r{l}"] = nc.declare_dram_parameter(f"bf2r{l}", [1, E], f32r, isOutput=False)
    dp["mt"] = nc.declare_dram_parameter("mt", [128, 2, E], f32r, isOutput=False)
    out_d = nc.declare_dram_parameter("out", [bpc, LH], f32, isOutput=True)

    with tile.TileContext(nc) as tc, ExitStack() as ctx:
        const = ctx.enter_context(tc.tile_pool(name="const", bufs=1))
        st = ctx.enter_context(tc.tile_pool(name="st", bufs=STBUFS))
        ps = ctx.enter_context(tc.tile_pool(name="ps", bufs=PSBUFS, space="PSUM"))

        # ---- constants / weights into SBUF ----
        ident = const.tile([128, 128], f32, tag="ident")
        make_identity(nc, ident)
        ident_r = const.tile([128, 128], f32r, tag="ident_r")
        nc.vector.tensor_copy(out=ident_r, in_=ident)
        ones_f = const.tile([1, 512], f32, tag="ones_f")
        nc.vector.memset(ones_f, 1.0)
        ones_r = const.tile([1, 512], f32r, tag="ones_r")
        nc.vector.tensor_copy(out=ones_r, in_=ones_f)
        ones16 = const.tile([1, 512], f16, tag="ones16")
        nc.vector.tensor_copy(out=ones16, in_=ones_f)
        eps = const.tile([128, 1], f32, tag="eps")
        nc.vector.memset(eps, 1e-5)
        onecol_f = const.tile([128, 2], f32, tag="onecol_f")
        nc.vector.memset(onecol_f, 1.0)
        onecol2_r = const.tile([128, 2], f32r, tag="onecol2_r")
        nc.vector.tensor_copy(out=onecol2_r, in_=onecol_f)

        cw = {}
        for nm, h in dp.items():
            t = const.tile(list(h.shape), h.dtype, tag=f"w_{nm}")
            nc.sync.dma_start(out=t, in_=h[:])
            cw[nm] = t

        mask_bt = const.tile([bpc, 304], f32, tag="mask_bt")
        nc.sync.dma_start(out=mask_bt, in_=maskbt_d[:])
        lv_bt = const.tile([bpc, LH], f32, tag="lv_bt")
        nc.sync.dma_start(out=lv_bt, in_=lvbt_d[:])

        compat_cols = const.tile([LH, max(bpc, 2)], f32, tag="compat_cols")

        def mm(out, lhsT, rhs, start, stop):
            nc.tensor.matmul(out, lhsT, rhs, start=start, stop=stop)

        ENGS2 = (nc.scalar, nc.vector)

        def ecopy(eng, out, in_):
            if eng is nc.scalar:
                nc.scalar.copy(out=out, in_=in_)
            else:
                eng.tensor_copy(out=out, in_=in_)

        def ps2(name, dt=f32):
            """Two-bank merged psum tile; each 512-f32 region holds one matmul."""
            return ps.tile([128, 2, 512], dt, tag="ps2", name=name, bufs=PSBUFS)

        def ps1(name, dt=f32):
            return ps.tile([128, 512], dt, tag="ps1", name=name, bufs=PS1BUFS)

        # ================= per batch element (generator, staged) =============
        def elem(b, lane):
            L = str(lane)
            rr = [lane]  # rotating engine picker for copies

            def pick():
                e = ENGS2[rr[0] % 2]
                rr[0] += 1
                return e

            # ---- S0: loads ----
            xT32 = st.tile([9, 304], f32, tag="xT32" + L, name="xT32")
            nc.sync.dma_start(out=xT32, in_=xt_d[b])
            xT = st.tile([9, 304], f16, tag="xT" + L, name="xT")
            nc.vector.tensor_copy(out=xT, in_=xT32)
            mcol = st.tile([128, 3, 2], f32r, tag="mcol" + L, name="mcol")
            nc.sync.dma_start(out=mcol, in_=mcol_d[b])
            yield

            # ---- S1: z1 = W1^T x + b1 ; lr = leakyrelu(z1) ----
            z1 = ps1("z1")
            for (c0, c1, ty), nf in zip(SEGS, (6, 8, 6)):
                mm(z1[:32, c0:c1], cw[f"w1_{ty}"], xT[:nf, c0:c1], True, False)
                mm(z1[:32, c0:c1], cw[f"b1r_{ty}"], ones16[:, :c1 - c0], False, True)
            small1 = st.tile([32, 304], f16, tag="small1" + L, name="small1")
            nc.scalar.activation(small1[:, :G], z1[:32, :G], AF.Identity, scale=0.01)
            lr = st.tile([32, 304], f16, tag="lr" + L, name="lr")
            nc.vector.tensor_tensor(out=lr[:, :G], in0=z1[:32, :G], in1=small1[:, :G],
                                    op=ALU.max)
            yield

            # ---- S2: h0T[e, g] = W2^T lr + b2 (direct transposed embed) ----
            hT = st.tile([128, 2, 304], f32r, tag="hT" + L, name="hT")
            hp = ps2("h0p")
            for m in range(2):
                for si, (c0, c1, ty) in enumerate(SEGS):
                    mm(hp[:, m, c0:c1], cw[f"w2_{ty}"][:, m * 128:(m + 1) * 128],
                       lr[:, c0:c1], True, False)
                    mm(hp[:, m, c0:c1], cw[f"b2r_{ty}"][:, m * 128:(m + 1) * 128],
                       ones16[:, :c1 - c0], False, True)
            ecopy(pick(), hT[:, :, :G], hp[:, :, :G])
            yield

            # ---- S3: h_nat = transpose(hT) ----
            h_nat = st.tile([128, 3, E], f32r, tag="hnat" + L, name="h_nat")
            tpn = ps2("tpn", f32r)
            for t in range(2):
                g0, g1 = GC2[t]
                for m in range(2):
                    nc.tensor.transpose(tpn[:g1 - g0, t, m * 128:(m + 1) * 128],
                                        hT[:, m, g0:g1], ident_r)
            ecopy(pick(), h_nat[:, 0:2, :], tpn[:, :, :E])
            g0, g1 = GC2[2]
            tpn2 = ps1("tpn2", f32r)
            for m in range(2):
                nc.tensor.transpose(tpn2[:g1 - g0, m * 128:(m + 1) * 128],
                                    hT[:, m, g0:g1], ident_r)
            ecopy(pick(), h_nat[:GC[2][1] - g0, 2, :], tpn2[:GC[2][1] - g0, :E])
            yield

            # ================= encoder layers =================
            for l in range(NL):
                # ---- S4: uT[e', g] = WQK^T hT ----
                u = st.tile([128, 2, 304], f32r, tag="uv" + L, name="u")
                up = ps2("up")
                for m in range(2):
                    for k in range(2):
                        mm(up[:, m, :VN], cw[f"wqk{l}"][:, k, m * 128:(m + 1) * 128],
                           hT[:, k, :VN], k == 0, k == 1)
                ecopy(pick(), u[:, :, :VN], up[:, :, :VN])
                yield

                # ---- S5: sT[k, q] = hT^T u ; aT = exp(sT) ----
                aT = st.tile([128, 4, 304], f32r, tag="ap" + L, name="aT")
                sp = ps2("sp")
                for t in range(2):
                    g0, g1 = GC[t]
                    for m in range(2):
                        mm(sp[:, t, :VN], hT[:, m, g0:g1], u[:, m, :VN],
                           m == 0, m == 1)
                nc.scalar.activation(aT[:, 0:2, :VN], sp[:, :, :VN], AF.Exp)
                g0, g1 = GC[2]
                sp2 = ps1("sp2")
                for m in range(2):
                    mm(sp2[:g1 - g0, :VN], hT[:, m, g0:g1], u[:, m, :VN],
                       m == 0, m == 1)
                nc.scalar.activation(aT[:g1 - g0, 2, :VN], sp2[:g1 - g0, :VN], AF.Exp)
                yield

                # ---- S6: rsum per q (column form) ; rinv = 1/rsum ----
                rs = ps.tile([128, 6], f32, tag="psS", name="rs", bufs=2)
                for tq in range(3):
                    q0, q1 = GC2[tq]
                    for tk in range(3):
                        k0, k1 = GC[tk]
                        ks = k1 - k0
                        mm(rs[:q1 - q0, 2 * tq:2 * tq + 2], aT[:ks, tk, q0:q1],
                           onecol2_r[:ks], tk == 0, tk == 2)
                rinv = st.tile([128, 3], f32, tag="rinv" + L, name="rinv")
                for tq in range(3):
                    q0, q1 = GC2[tq]
                    nc.vector.reciprocal(rinv[:q1 - q0, tq:tq + 1],
                                         rs[:q1 - q0, 2 * tq:2 * tq + 1])

                # ---- S7: avT[e, q] = h_nat^T aT ----
                avT = st.tile([128, 2, 304], f32r, tag="uv" + L, name="avT")
                ap_ = ps2("avp")
                for m in range(2):
                    for tk in range(3):
                        k0, k1 = GC[tk]
                        ks = k1 - k0
                        mm(ap_[:, m, :VN], h_nat[:ks, tk, m * 128:(m + 1) * 128],
                           aT[:ks, tk, :VN], tk == 0, tk == 2)
                ecopy(pick(), avT[:, :, :VN], ap_[:, :, :VN])
                yield

                # ---- S8: o = av @ WVO ; x1 = o*rinv + h (fused) ----
                x1 = st.tile([128, 3, E], f32, tag="x" + L, name="x1")
                op2 = ps2("op2")
                for t in range(2):
                    g0, g1 = GC[t]
                    for k in range(2):
                        mm(op2[:, t, :E], avT[:, k, g0:g1], cw[f"wvo{l}"][:, k, :],
                           k == 0, k == 1)
                op1 = ps1("op1")
                g0, g1 = GC[2]
                for k in range(2):
                    mm(op1[:g1 - g0, :E], avT[:, k, g0:g1], cw[f"wvo{l}"][:, k, :],
                       k == 0, k == 1)
                for t in range(2):
                    nc.vector.scalar_tensor_tensor(
                        out=x1[:, t, :], in0=op2[:, t, :E],
                        scalar=rinv[:, t:t + 1], in1=h_nat[:, t, :],
                        op0=ALU.mult, op1=ALU.add)
                gs = G - GC[2][0]
                nc.vector.scalar_tensor_tensor(
                    out=x1[:gs, 2, :], in0=op1[:gs, :E],
                    scalar=rinv[:gs, 2:3], in1=h_nat[:gs, 2, :],
                    op0=ALU.mult, op1=ALU.add)
                yield

                # ---- S9: LN1 stats ----
                mv1 = st.tile([128, 3, 2], f32, tag="mv1" + L, name="mv1")
                st6 = st.tile([128, 3, 6], f32, tag="st6" + L, name="st6", bufs=2)
                nc.vector.bn_stats(out=st6[:, 0:2, :], in_=x1[:, 0:2, :])
                nc.vector.bn_aggr(out=mv1[:, 0, :], in_=st6[:, 0, :])
                nc.vector.bn_aggr(out=mv1[:, 1, :], in_=st6[:, 1, :])
                gs = G - GC[2][0]
                nc.vector.bn_stats(out=st6[:gs, 2, :], in_=x1[:gs, 2, :])
                nc.vector.bn_aggr(out=mv1[:gs, 2, :], in_=st6[:gs, 2, :])
                lnv1 = st.tile([128, 3], f32, tag="lnv1" + L, name="lnv1")
                nc.scalar.activation(lnv1, mv1[:, :, 1], AF.Ln, bias=eps)
                rstd1 = st.tile([128, 3], f32, tag="rstd1" + L, name="rstd1")
                nc.scalar.activation(rstd1, lnv1, AF.Exp, scale=-0.5)
                yield

                # ---- S10: h1 = (x1 - m1) * rstd1 ----
                h1 = st.tile([128, 3, E], f32r, tag="h1" + L, name="h1")
                for t in range(3):
                    gs = GC[t][1] - GC[t][0]
                    nc.gpsimd.tensor_scalar(
                        out=h1[:gs, t, :], in0=x1[:gs, t, :],
                        scalar1=mv1[:gs, t, 0:1], scalar2=rstd1[:gs, t:t + 1],
                        op0=ALU.subtract, op1=ALU.mult)
                yield

                # ---- S11: h1T = transpose(h1) ----
                h1T = st.tile([128, 2, 304], f32r, tag="h1T" + L, name="h1T")
                tph = ps2("tph", f32r)
                for m in range(2):
                    for t in range(3):
                        g0, g1 = GC2[t]
                        gs = g1 - g0
                        nc.tensor.transpose(tph[:, m, g0:g1],
                                            h1[:gs, t, m * 128:(m + 1) * 128],
                                            ident_r[:gs, :gs])
                ecopy(pick(), h1T[:, :, :G], tph[:, :, :G])
                yield

                # ---- S12: f1 = relu(Wf1^T h1T + bf1) ----
                p = st.tile([128, 4, 304], f32r, tag="ap" + L, name="p")
                for half in range(2):
                    fp = ps2("fp")
                    for j in range(2):
                        m = half * 2 + j
                        for k in range(2):
                            mm(fp[:, j, :VN],
                               cw[f"wf1{l}"][:, k, m * 128:(m + 1) * 128],
                               h1T[:, k, :VN], k == 0, False)
                        mm(fp[:, j, :VN], cw[f"bf1r{l}"][:, m * 128:(m + 1) * 128],
                           ones_r[:, :VN], False, True)
                    nc.scalar.activation(p[:, 2 * half:2 * half + 2, :G],
                                         fp[:, :, :G], AF.Relu)
                yield

                # ---- S13: f2 + bf2 ; x2 = f2 + h1 (fused add) ----
                x2 = st.tile([128, 3, E], f32, tag="x" + L, name="x2")
                f2p = ps2("f2p")
                for t in range(2):
                    g0, g1 = GC[t]
                    for m in range(4):
                        mm(f2p[:, t, :E], p[:, m, g0:g1], cw[f"wf2{l}"][:, m, :],
                           m == 0, False)
                    mm(f2p[:, t, :E], ones_r[:, :g1 - g0], cw[f"bf2r{l}"],
                       False, True)
                nc.vector.tensor_tensor(out=x2[:, 0:2, :], in0=f2p[:, :, :E],
                                        in1=h1[:, 0:2, :], op=ALU.add)
                f2p1 = ps1("f2p1")
                g0, g1 = GC[2]
                gs = g1 - g0
                for m in range(4):
                    mm(f2p1[:gs, :E], p[:, m, g0:g1], cw[f"wf2{l}"][:, m, :],
                       m == 0, False)
                mm(f2p1[:gs, :E], ones_r[:, :gs], cw[f"bf2r{l}"], False, True)
                nc.vector.tensor_tensor(out=x2[:gs, 2, :], in0=f2p1[:gs, :E],
                                        in1=h1[:gs, 2, :], op=ALU.add)
                yield

                # ---- S14: LN2 stats ----
                mv2 = st.tile([128, 3, 2], f32, tag="mv2" + L, name="mv2")
                st6b = st.tile([128, 3, 6], f32, tag="st6" + L, name="st6b", bufs=2)
                nc.vector.bn_stats(out=st6b[:, 0:2, :], in_=x2[:, 0:2, :])
                nc.vector.bn_aggr(out=mv2[:, 0, :], in_=st6b[:, 0, :])
                nc.vector.bn_aggr(out=mv2[:, 1, :], in_=st6b[:, 1, :])
                gs = G - GC[2][0]
                nc.vector.bn_stats(out=st6b[:gs, 2, :], in_=x2[:gs, 2, :])
                nc.vector.bn_aggr(out=mv2[:gs, 2, :], in_=st6b[:gs, 2, :])
                lnv2 = st.tile([128, 3], f32, tag="lnv2" + L, name="lnv2")
                nc.scalar.activation(lnv2, mv2[:, :, 1], AF.Ln, bias=eps)
                rstd2 = st.tile([128, 3], f32, tag="rstd2" + L, name="rstd2")
                nc.scalar.activation(rstd2, lnv2, AF.Exp, scale=-0.5)
                yield

                # ---- S15: h2 = (x2 - m2) * rstd2 -> next h_nat ----
                h_nat = st.tile([128, 3, E], f32r, tag="hnat" + L, name="h_nat")
                for t in range(3):
                    gs = GC[t][1] - GC[t][0]
                    nc.gpsimd.tensor_scalar(
                        out=h_nat[:gs, t, :], in0=x2[:gs, t, :],
                        scalar1=mv2[:gs, t, 0:1], scalar2=rstd2[:gs, t:t + 1],
                        op0=ALU.subtract, op1=ALU.mult)
                yield

                # ---- S16: hT = transpose(h2) ----
                hT = st.tile([128, 2, 304], f32r, tag="hT" + L, name="hT")
                tpo = ps2("tpo", f32r)
                for m in range(2):
                    for t in range(3):
                        g0, g1 = GC2[t]
                        gs = g1 - g0
                        nc.tensor.transpose(tpo[:, m, g0:g1],
                                            h_nat[:gs, t, m * 128:(m + 1) * 128],
                                            ident_r[:gs, :gs])
                ecopy(pick(), hT[:, :, :G], tpo[:, :, :G])
                yield

            # ================= decoder =================
            ge_sb = st.tile([128, 2, 2], f32r, tag="ge" + L, name="ge_sb")
            for m in range(2):
                gep = ps.tile([128, 2], f32, tag="psS", name=f"gep{m}", bufs=2)
                for t in range(3):
                    g0, g1 = GC[t]
                    gs = g1 - g0
                    mm(gep, h_nat[:gs, t, m * 128:(m + 1) * 128],
                       mcol[:gs, t, :], t == 0, t == 2)
                nc.vector.tensor_copy(out=ge_sb[:, m, :], in_=gep)
            c_sb = st.tile([128, 2, 2], f32r, tag="c" + L, name="c_sb")
            for m in range(2):
                cp = ps.tile([128, 2], f32, tag="psS", name=f"cp{m}", bufs=2)
                for k in range(2):
                    mm(cp, cw["mt"][:, k, m * 128:(m + 1) * 128],
                       ge_sb[:, k, :], k == 0, k == 1)
                nc.vector.tensor_copy(out=c_sb[:, m, :], in_=cp)
            cc = ps.tile([LH, 2], f32, tag="psS", name="cc", bufs=2)
            for m in range(2):
                mm(cc, hT[:, m, IH:IH + LH], c_sb[:, m, :], m == 0, m == 1)
            nc.scalar.copy(out=compat_cols[:, b:b + 1], in_=cc[:, 0:1])
            yield

        # ---- drive the lanes (software pipeline, staggered starts) ----
        nb = nbp if nbp is not None else bpc
        nlanes = min(NWAY, nb)
        active = [elem(i, i) for i in range(nlanes)]
        nextb = nlanes
        rnd = 0
        while any(g is not None for g in active):
            for i in range(len(active)):
                g = active[i]
                if g is None or rnd < i * SKEW:
                    continue
                try:
                    next(g)
                except StopIteration:
                    if nextb < nb:
                        active[i] = elem(nextb, i)
                        nextb += 1
                    else:
                        active[i] = None
            rnd += 1

        # ================= batched tail =================
        vl = const.tile([bpc, 1], f32, tag="vl")
        nc.vector.reduce_sum(vl, mask_bt, axis=AX.X)
        ivl = const.tile([bpc, 1], f32, tag="ivl")
        nc.vector.reciprocal(ivl, vl)

        ctp = ps.tile([128, LH], f32, tag="psS", name="ctp", bufs=2)
        nc.tensor.transpose(ctp[:bpc, :LH], compat_cols[:, :bpc], ident[:LH, :LH])
        compat_sb = const.tile([bpc, LH], f32, tag="compat_sb")
        nc.vector.tensor_copy(compat_sb, ctp[:bpc, :LH])
        th = const.tile([bpc, LH], f32, tag="th")
        nc.scalar.activation(th, compat_sb, AF.Tanh, scale=ivl)
        ex = const.tile([bpc, LH], f32, tag="ex")
        es = const.tile([bpc, 1], f32, tag="es")
        nc.scalar.activation(ex, th, AF.Exp, scale=CLIP, accum_out=es)
        er = const.tile([bpc, 1], f32, tag="er")
        nc.vector.reciprocal(er, es)
        pm = const.tile([bpc, LH], f32, tag="pm")
        nc.vector.tensor_scalar_mul(pm, in0=ex, scalar1=er)
        nc.vector.tensor_tensor(out=pm, in0=pm, in1=lv_bt, op=mybir.AluOpType.mult)
        nc.vector.tensor_scalar_add(pm, in0=pm, scalar1=1e-20)
        rs2 = const.tile([bpc, 1], f32, tag="rs2")
        nc.vector.reduce_sum(rs2, pm, axis=AX.X)
        rr2 = const.tile([bpc, 1], f32, tag="rr2")
        nc.vector.reciprocal(rr2, rs2)
        ob = const.tile([bpc, LH], f32, tag="ob")
        nc.vector.tensor_scalar_mul(ob, in0=pm, scalar1=rr2)
        nc.sync.dma_start(out=out_d[:], in_=ob)

    nc.finalize()
    return nc


# ----------------------------------------------------------------------------
# public entry point
# ----------------------------------------------------------------------------
def kernel(**inputs):
    observation = np.asarray(inputs["observation"], np.float32)
    w = _prep_weights(inputs)

    from concourse.bass_utils import run_bass_kernel_spmd

    nc = _build(BPC)
    in_maps = []
    for i in range(NCORES):
        m = dict(w)
        m.update(_prep_obs(observation[i * BPC:(i + 1) * BPC]))
        in_maps.append(m)
    res = run_bass_kernel_spmd(nc, in_maps, list(range(NCORES)))
    out = np.concatenate([res.results[i]["out"] for i in range(NCORES)], axis=0)
    return out.astype(np.float32)


# revision 19
# speedup vs baseline: 1.7311x; 1.1008x over previous
"""Trainium2 Bass kernel for nn_AttentionModel (graph attention encoder + decoder).

Contract: kernel(**inputs) takes FULL unsharded numpy inputs (as produced by
reference.setup_inputs()) and returns the FULL [256, 100] float32 output.
Internally shards the batch (256) across 8 NeuronCores (32 each, pure data
parallel; weights replicated) and runs a fused Bass/Tile kernel per core.

v4: folded weights (Wq@Wk^T, Wv@Wo), transposed-score attention (no score
transposes), Ln/Exp-based layernorm rsqrt pinned to one activation table,
host-pretransposed observation, matmul-based decoder, merged two-bank PSUM
tiles with single wide drains, and N-way software pipelining across batch
elements to keep all engines fed.

Self-contained: hardcodes all shapes; no sibling imports.
"""

import os
import sys

for _p in ("/opt/trn_rl_repo", "/opt/pypackages"):
    if _p not in sys.path:
        sys.path.append(_p)

import numpy as np
from contextlib import ExitStack

# --- static architecture constants ---
B, IH, IL, LH, E, FFH, NL = 256, 200, 6, 100, 256, 512, 2
G = IH + LH + 1  # 301
CLIP = 10.0
SCALE = 1.0 / 16.0  # 1/sqrt(E)
NCORES = 8
BPC = B // NCORES  # 32 batch elements per core

NWAY = int(os.environ.get("KNWAY", "5"))      # software pipeline lanes
STBUFS = int(os.environ.get("KSTBUFS", "1"))  # sbuf bufs per tag
PSBUFS = int(os.environ.get("KPSBUFS", "2"))  # merged (2-bank) psum bufs
PS1BUFS = int(os.environ.get("KPS1BUFS", "2"))  # single-bank psum bufs
SKEW = int(os.environ.get("KSKEW", "0"))      # lane start stagger (rounds)

GC = [(0, 128), (128, 256), (256, 301)]   # g chunks (written ranges)
GC2 = [(0, 128), (128, 256), (256, 302)]  # even-padded ranges for fp32r
VN = 302  # padded moving width over the node axis
SEGS = [(0, IH, "i"), (IH, IH + LH, "l"), (IH + LH, G, "n")]  # embed type slices


# ----------------------------------------------------------------------------
# host-side weight packing
# ----------------------------------------------------------------------------
def _tf32(x):
    """Round fp32 array to tfloat32 (10 mantissa bits), round-to-nearest-even."""
    u = np.ascontiguousarray(x, np.float32).view(np.uint32)
    u = (u + 0x0FFF + ((u >> 13) & 1)) & np.uint32(0xFFFFE000)
    return u.view(np.float32)


def _pack_rows(m, nchunk):
    """[nchunk*128, N] -> [128, nchunk, N] with [:, k, :] = m[128k:128(k+1), :]"""
    return np.ascontiguousarray(
        np.stack([m[i * 128:(i + 1) * 128] for i in range(nchunk)], axis=1)
    ).astype(np.float32)


def _prep_weights(inp):
    w = {}
    # --- embedding (fp16) ---
    for t, wk1, bk1, wk2, bk2 in (
        ("i", "wi1", "bi1", "wi2", "bi2"),
        ("l", "wl1", "bl1", "wl2", "bl2"),
        ("n", "wn1", "bn1", "wn2", "bn2"),
    ):
        w[f"w1_{t}"] = np.asarray(inp[wk1], np.float16)
        w[f"b1r_{t}"] = np.asarray(inp[bk1], np.float16).reshape(1, 32)
        w[f"w2_{t}"] = np.asarray(inp[wk2], np.float16)           # [32, 256]
        w[f"b2r_{t}"] = np.asarray(inp[bk2], np.float16).reshape(1, E)
    # --- encoder layers (f32r) ---
    for l in range(NL):
        wqk = (np.asarray(inp["enc_wq"][l], np.float64) * SCALE) @ np.asarray(
            inp["enc_wk"][l], np.float64).T
        wvo = np.asarray(inp["enc_wv"][l], np.float64) @ np.asarray(
            inp["enc_wo"][l], np.float64)
        w[f"wqk{l}"] = _pack_rows(wqk.astype(np.float32), 2).astype(np.float16)
        w[f"wvo{l}"] = _tf32(_pack_rows(wvo.astype(np.float32), 2))   # rhs chunks
        w[f"wf1{l}"] = _pack_rows(inp["enc_wf1"][l], 2).astype(np.float16)
        w[f"bf1r{l}"] = _tf32(np.asarray(inp["enc_bf1"][l], np.float32).reshape(1, FFH))
        w[f"wf2{l}"] = _pack_rows(inp["enc_wf2"][l], 4).astype(np.float16)
        w[f"bf2r{l}"] = _tf32(np.asarray(inp["enc_bf2"][l], np.float32).reshape(1, E))
    # decoder fused matrix: compat = h_leaf . (M @ ge), M = Wpn_E @ Wfc.T
    MT = (np.asarray(inp["w_fc"], np.float64) @ np.asarray(
        inp["w_pn"], np.float64)[:, :E].T) * SCALE
    w["mt"] = _tf32(_pack_rows(MT.astype(np.float32), 2))             # [128,2,256] lhsT
    return w


def _prep_obs(observation):
    """Per-core observation-derived arrays (host-side layout transforms)."""
    obs = np.asarray(observation, np.float32)
    nb = obs.shape[0]
    # transposed features (values pre-rounded through fp16), g padded -> 304
    xt = np.zeros((nb, 9, 304), np.float32)
    xt[:, :, :G] = obs.transpose(0, 2, 1).astype(np.float16).astype(np.float32)
    # mask columns [nb, 128, 3, 2]: mcol[b, p, t, :] = mask[b, 128t + p]
    mask = obs[:, :, 8]
    mpad = np.zeros((nb, 384), np.float32)
    mpad[:, :G] = mask
    mcol = np.repeat(mpad.reshape(nb, 3, 128).transpose(0, 2, 1)[..., None], 2, axis=3)
    mcol = np.ascontiguousarray(mcol)
    # batched tail masks
    maskbt = np.zeros((nb, 304), np.float32)
    maskbt[:, :G] = mask
    lvbt = np.ascontiguousarray(mask[:, IH:IH + LH])
    return {"xt": xt, "mcol": mcol, "maskbt": maskbt, "lvbt": lvbt}


# ----------------------------------------------------------------------------
# numpy mirror of the device computation (for algebra validation)
# ----------------------------------------------------------------------------
def _numpy_mirror(observation, w):
    obs = np.asarray(observation, np.float32)
    nb = obs.shape[0]
    out = np.zeros((nb, LH), np.float32)

    def lrelu(x):
        return np.maximum(x, 0.01 * x)

    def ln(x):
        m = x.mean(-1, keepdims=True)
        v = x.var(-1, keepdims=True)
        return (x - m) * (v + 1e-5) ** -0.5

    for b in range(nb):
        xT = obs[b].T.astype(np.float16).astype(np.float32)  # [9, 301]
        h = np.zeros((G, E), np.float32)
        for (c0, c1, ty), nf in zip(SEGS, (6, 8, 6)):
            z = xT[:nf, c0:c1].T @ w[f"w1_{ty}"].astype(np.float32) \
                + w[f"b1r_{ty}"].astype(np.float32)
            h[c0:c1] = lrelu(z) @ w[f"w2_{ty}"].astype(np.float32) \
                + w[f"b2r_{ty}"].astype(np.float32)

        for l in range(NL):
            wqk = np.concatenate([w[f"wqk{l}"][:, 0], w[f"wqk{l}"][:, 1]], 0)
            wvo = np.concatenate([w[f"wvo{l}"][:, 0], w[f"wvo{l}"][:, 1]], 0)
            wf1 = np.concatenate([w[f"wf1{l}"][:, 0], w[f"wf1{l}"][:, 1]], 0)
            wf2 = np.concatenate([w[f"wf2{l}"][:, k] for k in range(4)], 0)
            u = h @ wqk
            sT = h @ u.T                      # sT[k, q]
            aT = np.exp(sT)
            rs = aT.sum(0)                    # per q
            o = (aT.T @ h) @ wvo              # [q, e]
            x1 = o / rs[:, None] + h
            h1 = ln(x1)
            p = np.maximum(h1 @ wf1 + w[f"bf1r{l}"], 0.0)
            x2 = p @ wf2 + w[f"bf2r{l}"] + h1
            h = ln(x2)

        mask = obs[b, :, 8]
        ge = (h * mask[:, None]).sum(0)       # unnormalized
        MT = np.concatenate([w["mt"][:, 0], w["mt"][:, 1]], 0)
        c = MT.T @ ge
        compat = h[IH:IH + LH] @ c            # unmasked leaf compat
        vlen = mask.sum()
        logits = np.tanh(compat / vlen) * CLIP
        ee = np.exp(logits)
        pp = ee / ee.sum()
        lv = mask[IH:IH + LH]
        masked = pp * lv + 1e-20
        out[b] = masked / masked.sum()
    return out


# ----------------------------------------------------------------------------
# the Bass/Tile kernel
# ----------------------------------------------------------------------------
def _build(bpc, nbp=None):
    import concourse.bass as bass
    import concourse.mybir as mybir
    import concourse.tile as tile
    from concourse import bacc
    from concourse.masks import make_identity

    f32 = mybir.dt.float32
    f32r = mybir.dt.float32r
    f16 = mybir.dt.float16
    AF = mybir.ActivationFunctionType
    ALU = mybir.AluOpType
    AX = mybir.AxisListType

    # Steer the act-table chooser: greedy first-match would pick tables that
    # split Ln and Exp, reloading on every layernorm. Present a view where the
    # shared funcs resolve only to natural_log_exp_and_others (indices are
    # preserved, so emitted act_func_set_ids stay valid for act_info.json).
    import concourse.hw_specs as _hw_specs
    _real_gat = _hw_specs.get_activation_tables

    def _patched_gat(arch):
        t = dict(_real_gat(arch))
        keep = "natural_log_exp_and_others"
        shared = {
            AF.Exp, AF.Ln, AF.Identity, AF.Copy, AF.Relu, AF.Prelu, AF.Square,
        }
        out = {}
        for name, funcs in t.items():
            out[name] = set(funcs) if name == keep else set(funcs) - shared
        return out

    bacc.get_activation_tables = _patched_gat

    nc = bacc.Bacc(None, target_bir_lowering=False)

    xt_d = nc.declare_dram_parameter("xt", [bpc, 9, 304], f32, isOutput=False)
    mcol_d = nc.declare_dram_parameter("mcol", [bpc, 128, 3, 2], f32r, isOutput=False)
    maskbt_d = nc.declare_dram_parameter("maskbt", [bpc, 304], f32, isOutput=False)
    lvbt_d = nc.declare_dram_parameter("lvbt", [bpc, LH], f32, isOutput=False)
    dp = {}
    for t in "iln":
        nf = 8 if t == "l" else 6
        dp[f"w1_{t}"] = nc.declare_dram_parameter(f"w1_{t}", [nf, 32], f16, isOutput=False)
        dp[f"b1r_{t}"] = nc.declare_dram_parameter(f"b1r_{t}", [1, 32], f16, isOutput=False)
        dp[f"w2_{t}"] = nc.declare_dram_parameter(f"w2_{t}", [32, E], f16, isOutput=False)
        dp[f"b2r_{t}"] = nc.declare_dram_parameter(f"b2r_{t}", [1, E], f16, isOutput=False)
    for l in range(NL):
        dp[f"wqk{l}"] = nc.declare_dram_parameter(f"wqk{l}", [128, 2, E], f16, isOutput=False)
        dp[f"wvo{l}"] = nc.declare_dram_parameter(f"wvo{l}", [128, 2, E], f32r, isOutput=False)
        dp[f"wf1{l}"] = nc.declare_dram_parameter(f"wf1{l}", [128, 2, FFH], f16, isOutput=False)
        dp[f"bf1r{l}"] = nc.declare_dram_parameter(f"bf1r{l}", [1, FFH], f32r, isOutput=False)
        dp[f"wf2{l}"] = nc.declare_dram_parameter(f"wf2{l}", [128, 4, E], f16, isOutput=False)
        dp[f"bf2r{l}"] = nc.declare_dram_parameter(f"bf2r{l}", [1, E], f32r, isOutput=False)
    dp["mt"] = nc.declare_dram_parameter("mt", [128, 2, E], f32r, isOutput=False)
    out_d = nc.declare_dram_parameter("out", [bpc, LH], f32, isOutput=True)

    with tile.TileContext(nc) as tc, ExitStack() as ctx:
        const = ctx.enter_context(tc.tile_pool(name="const", bufs=1))
        st = ctx.enter_context(tc.tile_pool(name="st", bufs=STBUFS))
        ps = ctx.enter_context(tc.tile_pool(name="ps", bufs=PSBUFS, space="PSUM"))

        # ---- constants / weights into SBUF ----
        ident = const.tile([128, 128], f32, tag="ident")
        make_identity(nc, ident)
        ident_r = const.tile([128, 128], f32r, tag="ident_r")
        nc.vector.tensor_copy(out=ident_r, in_=ident)
        ones_f = const.tile([1, 512], f32, tag="ones_f")
        nc.vector.memset(ones_f, 1.0)
        ones_r = const.tile([1, 512], f32r, tag="ones_r")
        nc.vector.tensor_copy(out=ones_r, in_=ones_f)
        ones16 = const.tile([1, 512], f16, tag="ones16")
        nc.vector.tensor_copy(out=ones16, in_=ones_f)
        eps = const.tile([128, 1], f32, tag="eps")
        nc.vector.memset(eps, 1e-5)
        onecol_f = const.tile([128, 2], f32, tag="onecol_f")
        nc.vector.memset(onecol_f, 1.0)
        onecol2_r = const.tile([128, 2], f32r, tag="onecol2_r")
        nc.vector.tensor_copy(out=onecol2_r, in_=onecol_f)

        cw = {}
        for nm, h in dp.items():
            t = const.tile(list(h.shape), h.dtype, tag=f"w_{nm}")
            nc.sync.dma_start(out=t, in_=h[:])
            cw[nm] = t

        mask_bt = const.tile([bpc, 304], f32, tag="mask_bt")
        nc.sync.dma_start(out=mask_bt, in_=maskbt_d[:])
        lv_bt = const.tile([bpc, LH], f32, tag="lv_bt")
        nc.sync.dma_start(out=lv_bt, in_=lvbt_d[:])

        compat_cols = const.tile([LH, max(bpc, 2)], f32, tag="compat_cols")

        def mm(out, lhsT, rhs, start, stop):
            nc.tensor.matmul(out, lhsT, rhs, start=start, stop=stop)

        ENGS2 = (nc.scalar, nc.vector)

        def ecopy(eng, out, in_):
            if eng is nc.scalar:
                nc.scalar.copy(out=out, in_=in_)
            else:
                eng.tensor_copy(out=out, in_=in_)

        def ps2(name, dt=f32):
            """Two-bank merged psum tile; each 512-f32 region holds one matmul."""
            return ps.tile([128, 2, 512], dt, tag="ps2", name=name, bufs=PSBUFS)

        def ps1(name, dt=f32):
            return ps.tile([128, 512], dt, tag="ps1", name=name, bufs=PS1BUFS)

        # ================= per batch element (generator, staged) =============
        def elem(b, lane):
            L = str(lane)
            rr = [lane]  # rotating engine picker for copies

            def pick():
                e = ENGS2[rr[0] % 2]
                rr[0] += 1
                return e

            # ---- S0: loads ----
            xT32 = st.tile([9, 304], f32, tag="xT32" + L, name="xT32")
            nc.sync.dma_start(out=xT32, in_=xt_d[b])
            xT = st.tile([9, 304], f16, tag="xT" + L, name="xT")
            nc.vector.tensor_copy(out=xT, in_=xT32)
            mcol = st.tile([128, 3, 2], f32r, tag="mcol" + L, name="mcol")
            nc.sync.dma_start(out=mcol, in_=mcol_d[b])
            yield

            # ---- S1: z1 = W1^T x + b1 ; lr = leakyrelu(z1) ----
            z1 = ps1("z1")
            for (c0, c1, ty), nf in zip(SEGS, (6, 8, 6)):
                mm(z1[:32, c0:c1], cw[f"w1_{ty}"], xT[:nf, c0:c1], True, False)
                mm(z1[:32, c0:c1], cw[f"b1r_{ty}"], ones16[:, :c1 - c0], False, True)
            small1 = st.tile([32, 304], f16, tag="small1" + L, name="small1")
            nc.scalar.activation(small1[:, :G], z1[:32, :G], AF.Identity, scale=0.01)
            lr = st.tile([32, 304], f16, tag="lr" + L, name="lr")
            nc.vector.tensor_tensor(out=lr[:, :G], in0=z1[:32, :G], in1=small1[:, :G],
                                    op=ALU.max)
            yield

            # ---- S2: h0T[e, g] = W2^T lr + b2 (direct transposed embed) ----
            hT = st.tile([128, 2, 304], f16, tag="hT" + L, name="hT")
            hp = ps2("h0p")
            for m in range(2):
                for si, (c0, c1, ty) in enumerate(SEGS):
                    mm(hp[:, m, c0:c1], cw[f"w2_{ty}"][:, m * 128:(m + 1) * 128],
                       lr[:, c0:c1], True, False)
                    mm(hp[:, m, c0:c1], cw[f"b2r_{ty}"][:, m * 128:(m + 1) * 128],
                       ones16[:, :c1 - c0], False, True)
            ecopy(pick(), hT[:, :, :G], hp[:, :, :G])
            yield

            # ---- S3: h_nat = transpose(hT) ----
            h_nat = st.tile([128, 3, E], f32r, tag="hnat" + L, name="h_nat")
            tpn = ps2("tpn", f16)
            for t in range(2):
                g0, g1 = GC2[t]
                for m in range(2):
                    nc.tensor.transpose(tpn[:g1 - g0, t, m * 128:(m + 1) * 128],
                                        hT[:, m, g0:g1], ident_r)
            ecopy(pick(), h_nat[:, 0:2, :], tpn[:, :, :E])
            g0, g1 = GC2[2]
            tpn2 = ps1("tpn2", f16)
            for m in range(2):
                nc.tensor.transpose(tpn2[:g1 - g0, m * 128:(m + 1) * 128],
                                    hT[:, m, g0:g1], ident_r)
            ecopy(pick(), h_nat[:GC[2][1] - g0, 2, :], tpn2[:GC[2][1] - g0, :E])
            yield

            # ================= encoder layers =================
            for l in range(NL):
                # ---- S4: uT[e', g] = WQK^T hT ----
                u = st.tile([128, 2, 304], f16, tag="uv" + L, name="u")
                up = ps2("up")
                for m in range(2):
                    for k in range(2):
                        mm(up[:, m, :VN], cw[f"wqk{l}"][:, k, m * 128:(m + 1) * 128],
                           hT[:, k, :VN], k == 0, k == 1)
                ecopy(pick(), u[:, :, :VN], up[:, :, :VN])
                yield

                # ---- S5: sT[k, q] = hT^T u ; aT = exp(sT) ----
                aT = st.tile([128, 4, 304], f32r, tag="ap" + L, name="aT")
                sp = ps2("sp")
                for t in range(2):
                    g0, g1 = GC[t]
                    for m in range(2):
                        mm(sp[:, t, :VN], hT[:, m, g0:g1], u[:, m, :VN],
                           m == 0, m == 1)
                nc.scalar.activation(aT[:, 0:2, :VN], sp[:, :, :VN], AF.Exp)
                g0, g1 = GC[2]
                sp2 = ps1("sp2")
                for m in range(2):
                    mm(sp2[:g1 - g0, :VN], hT[:, m, g0:g1], u[:, m, :VN],
                       m == 0, m == 1)
                nc.scalar.activation(aT[:g1 - g0, 2, :VN], sp2[:g1 - g0, :VN], AF.Exp)
                yield

                # ---- S6: rsum per q (column form) ; rinv = 1/rsum ----
                rs = ps.tile([128, 6], f32, tag="psS", name="rs", bufs=2)
                for tq in range(3):
                    q0, q1 = GC2[tq]
                    for tk in range(3):
                        k0, k1 = GC[tk]
                        ks = k1 - k0
                        mm(rs[:q1 - q0, 2 * tq:2 * tq + 2], aT[:ks, tk, q0:q1],
                           onecol2_r[:ks], tk == 0, tk == 2)
                rinv = st.tile([128, 3], f32, tag="rinv" + L, name="rinv")
                for tq in range(3):
                    q0, q1 = GC2[tq]
                    nc.vector.reciprocal(rinv[:q1 - q0, tq:tq + 1],
                                         rs[:q1 - q0, 2 * tq:2 * tq + 1])

                # ---- S7: avT[e, q] = h_nat^T aT ----
                avT = st.tile([128, 2, 304], f32r, tag="uv" + L, name="avT")
                ap_ = ps2("avp")
                for m in range(2):
                    for tk in range(3):
                        k0, k1 = GC[tk]
                        ks = k1 - k0
                        mm(ap_[:, m, :VN], h_nat[:ks, tk, m * 128:(m + 1) * 128],
                           aT[:ks, tk, :VN], tk == 0, tk == 2)
                ecopy(pick(), avT[:, :, :VN], ap_[:, :, :VN])
                yield

                # ---- S8: o = av @ WVO ; x1 = o*rinv + h (fused) ----
                x1 = st.tile([128, 3, E], f32, tag="x" + L, name="x1")
                op2 = ps2("op2")
                for t in range(2):
                    g0, g1 = GC[t]
                    for k in range(2):
                        mm(op2[:, t, :E], avT[:, k, g0:g1], cw[f"wvo{l}"][:, k, :],
                           k == 0, k == 1)
                op1 = ps1("op1")
                g0, g1 = GC[2]
                for k in range(2):
                    mm(op1[:g1 - g0, :E], avT[:, k, g0:g1], cw[f"wvo{l}"][:, k, :],
                       k == 0, k == 1)
                for t in range(2):
                    nc.vector.scalar_tensor_tensor(
                        out=x1[:, t, :], in0=op2[:, t, :E],
                        scalar=rinv[:, t:t + 1], in1=h_nat[:, t, :],
                        op0=ALU.mult, op1=ALU.add)
                gs = G - GC[2][0]
                nc.vector.scalar_tensor_tensor(
                    out=x1[:gs, 2, :], in0=op1[:gs, :E],
                    scalar=rinv[:gs, 2:3], in1=h_nat[:gs, 2, :],
                    op0=ALU.mult, op1=ALU.add)
                yield

                # ---- S9: LN1 stats ----
                mv1 = st.tile([128, 3, 2], f32, tag="mv1" + L, name="mv1")
                st6 = st.tile([128, 3, 6], f32, tag="st6" + L, name="st6", bufs=2)
                nc.vector.bn_stats(out=st6[:, 0:2, :], in_=x1[:, 0:2, :])
                nc.vector.bn_aggr(out=mv1[:, 0, :], in_=st6[:, 0, :])
                nc.vector.bn_aggr(out=mv1[:, 1, :], in_=st6[:, 1, :])
                gs = G - GC[2][0]
                nc.vector.bn_stats(out=st6[:gs, 2, :], in_=x1[:gs, 2, :])
                nc.vector.bn_aggr(out=mv1[:gs, 2, :], in_=st6[:gs, 2, :])
                lnv1 = st.tile([128, 3], f32, tag="lnv1" + L, name="lnv1")
                nc.scalar.activation(lnv1, mv1[:, :, 1], AF.Ln, bias=eps)
                rstd1 = st.tile([128, 3], f32, tag="rstd1" + L, name="rstd1")
                nc.scalar.activation(rstd1, lnv1, AF.Exp, scale=-0.5)
                yield

                # ---- S10: h1 = (x1 - m1) * rstd1 ----
                h1 = st.tile([128, 3, E], f16, tag="h1" + L, name="h1")
                for t in range(3):
                    gs = GC[t][1] - GC[t][0]
                    nc.gpsimd.tensor_scalar(
                        out=h1[:gs, t, :], in0=x1[:gs, t, :],
                        scalar1=mv1[:gs, t, 0:1], scalar2=rstd1[:gs, t:t + 1],
                        op0=ALU.subtract, op1=ALU.mult)
                yield

                # ---- S11: h1T = transpose(h1) ----
                h1T = st.tile([128, 2, 304], f16, tag="h1T" + L, name="h1T")
                tph = ps2("tph", f16)
                for m in range(2):
                    for t in range(3):
                        g0, g1 = GC2[t]
                        gs = g1 - g0
                        nc.tensor.transpose(tph[:, m, g0:g1],
                                            h1[:gs, t, m * 128:(m + 1) * 128],
                                            ident_r[:gs, :gs])
                ecopy(pick(), h1T[:, :, :G], tph[:, :, :G])
                yield

                # ---- S12: f1 = relu(Wf1^T h1T + bf1) ----
                p = st.tile([128, 4, 304], f16, tag="ap" + L, name="p")
                for half in range(2):
                    fp = ps2("fp")
                    for j in range(2):
                        m = half * 2 + j
                        for k in range(2):
                            mm(fp[:, j, :VN],
                               cw[f"wf1{l}"][:, k, m * 128:(m + 1) * 128],
                               h1T[:, k, :VN], k == 0, False)
                        mm(fp[:, j, :VN], cw[f"bf1r{l}"][:, m * 128:(m + 1) * 128],
                           ones_r[:, :VN], False, True)
                    nc.scalar.activation(p[:, 2 * half:2 * half + 2, :G],
                                         fp[:, :, :G], AF.Relu)
                yield

                # ---- S13: f2 + bf2 ; x2 = f2 + h1 (fused add) ----
                x2 = st.tile([128, 3, E], f32, tag="x" + L, name="x2")
                f2p = ps2("f2p")
                for t in range(2):
                    g0, g1 = GC[t]
                    for m in range(4):
                        mm(f2p[:, t, :E], p[:, m, g0:g1], cw[f"wf2{l}"][:, m, :],
                           m == 0, False)
                    mm(f2p[:, t, :E], ones_r[:, :g1 - g0], cw[f"bf2r{l}"],
                       False, True)
                nc.vector.tensor_tensor(out=x2[:, 0:2, :], in0=f2p[:, :, :E],
                                        in1=h1[:, 0:2, :], op=ALU.add)
                f2p1 = ps1("f2p1")
                g0, g1 = GC[2]
                gs = g1 - g0
                for m in range(4):
                    mm(f2p1[:gs, :E], p[:, m, g0:g1], cw[f"wf2{l}"][:, m, :],
                       m == 0, False)
                mm(f2p1[:gs, :E], ones_r[:, :gs], cw[f"bf2r{l}"], False, True)
                nc.vector.tensor_tensor(out=x2[:gs, 2, :], in0=f2p1[:gs, :E],
                                        in1=h1[:gs, 2, :], op=ALU.add)
                yield

                # ---- S14: LN2 stats ----
                mv2 = st.tile([128, 3, 2], f32, tag="mv2" + L, name="mv2")
                st6b = st.tile([128, 3, 6], f32, tag="st6" + L, name="st6b", bufs=2)
                nc.vector.bn_stats(out=st6b[:, 0:2, :], in_=x2[:, 0:2, :])
                nc.vector.bn_aggr(out=mv2[:, 0, :], in_=st6b[:, 0, :])
                nc.vector.bn_aggr(out=mv2[:, 1, :], in_=st6b[:, 1, :])
                gs = G - GC[2][0]
                nc.vector.bn_stats(out=st6b[:gs, 2, :], in_=x2[:gs, 2, :])
                nc.vector.bn_aggr(out=mv2[:gs, 2, :], in_=st6b[:gs, 2, :])
                lnv2 = st.tile([128, 3], f32, tag="lnv2" + L, name="lnv2")
                nc.scalar.activation(lnv2, mv2[:, :, 1], AF.Ln, bias=eps)
                rstd2 = st.tile([128, 3], f32, tag="rstd2" + L, name="rstd2")
                nc.scalar.activation(rstd2, lnv2, AF.Exp, scale=-0.5)
                yield

                # ---- S15: h2 = (x2 - m2) * rstd2 -> next h_nat ----
                h_nat = st.tile([128, 3, E], f32r, tag="hnat" + L, name="h_nat")
                for t in range(3):
                    gs = GC[t][1] - GC[t][0]
                    nc.gpsimd.tensor_scalar(
                        out=h_nat[:gs, t, :], in0=x2[:gs, t, :],
                        scalar1=mv2[:gs, t, 0:1], scalar2=rstd2[:gs, t:t + 1],
                        op0=ALU.subtract, op1=ALU.mult)
                yield

                # ---- S16: hT = transpose(h2) ----
                hT = st.tile([128, 2, 304], f16, tag="hT" + L, name="hT")
                tpo = ps2("tpo", f32r)
                for m in range(2):
                    for t in range(3):
                        g0, g1 = GC2[t]
                        gs = g1 - g0
                        nc.tensor.transpose(tpo[:, m, g0:g1],
                                            h_nat[:gs, t, m * 128:(m + 1) * 128],
                                            ident_r[:gs, :gs])
                ecopy(pick(), hT[:, :, :G], tpo[:, :, :G])
                yield

            # ================= decoder =================
            ge_sb = st.tile([128, 2, 2], f32r, tag="ge" + L, name="ge_sb")
            for m in range(2):
                gep = ps.tile([128, 2], f32, tag="psS", name=f"gep{m}", bufs=2)
                for t in range(3):
                    g0, g1 = GC[t]
                    gs = g1 - g0
                    mm(gep, h_nat[:gs, t, m * 128:(m + 1) * 128],
                       mcol[:gs, t, :], t == 0, t == 2)
                nc.vector.tensor_copy(out=ge_sb[:, m, :], in_=gep)
            c_sb = st.tile([128, 2, 2], f16, tag="c" + L, name="c_sb")
            for m in range(2):
                cp = ps.tile([128, 2], f32, tag="psS", name=f"cp{m}", bufs=2)
                for k in range(2):
                    mm(cp, cw["mt"][:, k, m * 128:(m + 1) * 128],
                       ge_sb[:, k, :], k == 0, k == 1)
                nc.vector.tensor_copy(out=c_sb[:, m, :], in_=cp)
            cc = ps.tile([LH, 2], f32, tag="psS", name="cc", bufs=2)
            for m in range(2):
                mm(cc, hT[:, m, IH:IH + LH], c_sb[:, m, :], m == 0, m == 1)
            nc.scalar.copy(out=compat_cols[:, b:b + 1], in_=cc[:, 0:1])
            yield

        # ---- drive the lanes (software pipeline, staggered starts) ----
        nb = nbp if nbp is not None else bpc
        nlanes = min(NWAY, nb)
        active = [elem(i, i) for i in range(nlanes)]
        nextb = nlanes
        rnd = 0
        while any(g is not None for g in active):
            for i in range(len(active)):
                g = active[i]
                if g is None or rnd < i * SKEW:
                    continue
                try:
                    next(g)
                except StopIteration:
                    if nextb < nb:
                        active[i] = elem(nextb, i)
                        nextb += 1
                    else:
                        active[i] = None
            rnd += 1

        # ================= batched tail =================
        vl = const.tile([bpc, 1], f32, tag="vl")
        nc.vector.reduce_sum(vl, mask_bt, axis=AX.X)
        ivl = const.tile([bpc, 1], f32, tag="ivl")
        nc.vector.reciprocal(ivl, vl)

        ctp = ps.tile([128, LH], f32, tag="psS", name="ctp", bufs=2)
        nc.tensor.transpose(ctp[:bpc, :LH], compat_cols[:, :bpc], ident[:LH, :LH])
        compat_sb = const.tile([bpc, LH], f32, tag="compat_sb")
        nc.vector.tensor_copy(compat_sb, ctp[:bpc, :LH])
        th = const.tile([bpc, LH], f32, tag="th")
        nc.scalar.activation(th, compat_sb, AF.Tanh, scale=ivl)
        ex = const.tile([bpc, LH], f32, tag="ex")
        es = const.tile([bpc, 1], f32, tag="es")
        nc.scalar.activation(ex, th, AF.Exp, scale=CLIP, accum_out=es)
        er = const.tile([bpc, 1], f32, tag="er")
        nc.vector.reciprocal(er, es)
        pm = const.tile([bpc, LH], f32, tag="pm")
        nc.vector.tensor_scalar_mul(pm, in0=ex, scalar1=er)
        nc.vector.tensor_tensor(out=pm, in0=pm, in1=lv_bt, op=mybir.AluOpType.mult)
        nc.vector.tensor_scalar_add(pm, in0=pm, scalar1=1e-20)
        rs2 = const.tile([bpc, 1], f32, tag="rs2")
        nc.vector.reduce_sum(rs2, pm, axis=AX.X)
        rr2 = const.tile([bpc, 1], f32, tag="rr2")
        nc.vector.reciprocal(rr2, rs2)
        ob = const.tile([bpc, LH], f32, tag="ob")
        nc.vector.tensor_scalar_mul(ob, in0=pm, scalar1=rr2)
        nc.sync.dma_start(out=out_d[:], in_=ob)

    nc.finalize()
    return nc


# ----------------------------------------------------------------------------
# public entry point
# ----------------------------------------------------------------------------
def kernel(**inputs):
    observation = np.asarray(inputs["observation"], np.float32)
    w = _prep_weights(inputs)

    from concourse.bass_utils import run_bass_kernel_spmd

    nc = _build(BPC)
    in_maps = []
    for i in range(NCORES):
        m = dict(w)
        m.update(_prep_obs(observation[i * BPC:(i + 1) * BPC]))
        in_maps.append(m)
    res = run_bass_kernel_spmd(nc, in_maps, list(range(NCORES)))
    out = np.concatenate([res.results[i]["out"] for i in range(NCORES)], axis=0)
    return out.astype(np.float32)
